# revision 25
# baseline (speedup 1.0000x reference)
"""GAT layer (nn_GAT_21930103013469) on 8 trn2 NeuronCores — v3.

Reference (per batch b):
    Wh  = h @ W                                   [N, F]
    s1  = Wh @ a1,  s2 = Wh @ a2                  [N]
    e   = leakyrelu(s1[:,None] + s2[None,:], 0.2) [N, N]
    att = softmax(where(adj>0, e, -9e15), axis=1)   (normalized over rows i)
    out = elu(att @ Wh)

Data parallel over B=16 (2 batches per core). Attention is computed
TRANSPOSED (PT[j, i], partition j, free i) so the softmax reduction
(over i) is the ACT Exp pass's accum_out and the output matmul
out^T[o, i] = sum_j V[j, o] * PT[j, i] contracts j on partitions.

v3 structure (vs the 218 us baseline):
  - adjT is HOST-prepped int8 in {-128, 0}; the mask rides the logit
    (u = s1[i] + s2[j] + adjm, exp suppresses masked entries by
    e^-25.6 after the leaky slope). 4x less DMA than int32.
  - per unit the whole logit assembly is ONE custom DVE op
        w2 = lrelu(s1b + adjm + s2col)
    followed by ONE ACT Exp (bias=-SHIFT, accum_out=z). The per-elem
    work is 1 DVE + 1 ACT pass (baseline: 1 DVE + 2 ACT equivalents).
  - h is HOST-transposed/cast to f16 (no PE transposes), W cast f16
    with VSCALE folded in, c = W @ a computed on host (F*F*2 MACs,
    0.1% of the flops).
  - elu epilogue: elu(x) = min(exp(x)-1, relu(x)): ACT exp + one
    fused DVE op; one output DMA per (batch, ot) row block.
  - Pool engine (ISA-limited to copies here) takes the PSUM->SBUF
    copies (wh, s1b, sT); sync engine takes all input DMA kicks.
"""
import sys

sys.path.insert(0, "/opt/trn_rl_repo")

import contextlib

import numpy as np

import concourse.bacc as bacc
import concourse.tile as tile
from concourse import mybir
from concourse.bass_utils import run_bass_kernel_spmd

B, N, F = 16, 2048, 256
NCORES = 8
BPC = B // NCORES          # batches per core
NT = N // 128              # 16 j tiles
FT = F // 128              # 2 fout tiles
ALPHA = 0.2
SHIFT = 10.0               # PT = exp(lrelu(u) - SHIFT)
VSCALE = 8.0               # folded into W on host; out = PV / VSCALE
MASKC = 128                # adjm = (adj - 1) * MASKC  in {-128, 0}
TRAIL0 = 5                 # pv trails pt production (batch 0): prep(1) is
                           # dripped into stream(0) and all its PSUM tiles
                           # must be emitted before pv8(0, 0) claims 8 banks
TRAIL1 = 4                 # batch 1 trails: elu(0) frees PSUM banks first;
                           # must stay >= 4 so v(1, jt-TRAIL1) is emitted
                           # (recip_v lags the exps by ~3 units)

f32, f32r, f16, i8 = (mybir.dt.float32, mybir.dt.float32r,
                      mybir.dt.float16, mybir.dt.int8)
AF = mybir.ActivationFunctionType
OP = mybir.AluOpType

# how many of the 32 V-scale passes go to ACT (balance DVE vs ACT)
V_ON_ACT_MOD = 8           # jt % MOD == 0 -> ACT copy-with-scale


# ---------------------------------------------------------------------------
# Custom fused DVE ops, registered at import into concourse.dve_ops'
# tables (same machinery as the production ops; the per-NEFF DVE table
# is generated from these specs by bass_utils.dve_table_for_ops).
# ---------------------------------------------------------------------------


def _register_ops():
    import concourse.dve_ops as dve_ops_mod
    from concourse.dve_ops import DveOp
    from concourse.dve_spec import C0, C1, C2, Spec, Src0, Src1
    from concourse.dve_spec import lower as dve_lower
    from concourse.dve_spec import maxx, minn, relu
    from concourse.dve_uop import DveOpSpec

    def mk(name, spec):
        for op in dve_ops_mod.OPS:
            if op.name == name:
                return op
        row = max(dve_ops_mod._SUB_OPCODE_FOR_NAME.values()) + 1
        shas = {
            ver: DveOpSpec(name=name, opcode=row,
                           uops=dve_lower(spec, ver=ver),
                           rd1_en=True).sha(ver)
            for ver in ("v3", "v4")
        }
        op = DveOp(name, spec, subdim=False, uops_sha=shas)
        dve_ops_mod.OPS.append(op)
        dve_ops_mod.CUSTOM_DVE_SPECS[name] = spec
        dve_ops_mod._SUB_OPCODE_FOR_NAME[name] = row
        return op

    # out = lrelu(in0 + in1 + s0), slope imm2
    y = (Src0 + Src1) + C0

    def lrelu_ref(in0, in1, s0, s1, imm2):
        yy = in0.astype(np.float32) + in1 + s0
        return np.maximum(yy, yy * imm2).astype(np.float32)

    lrelu_op = mk("LRELU_MADD_ANT",
                  Spec(body=maxx(y, y * C2), reference=lrelu_ref))

    # out = min(in0 - s1, relu(in1 * s0)):  elu(x) with in0 = exp(x),
    # in1 = VSCALE*x, s0 = 1/VSCALE, s1 = 1.
    def elu_ref(in0, in1, s0, s1, imm2):
        r = np.maximum(np.nan_to_num(in1.astype(np.float32) * s0,
                                     nan=0.0, posinf=np.inf,
                                     neginf=-np.inf), 0)
        return np.minimum(in0.astype(np.float32) - s1, r).astype(np.float32)

    elu_op = mk("ELU_TAIL_ANT",
                Spec(body=minn(Src0 - C1, relu(Src1 * C0)),
                     reference=elu_ref))
    return lrelu_op, elu_op


LRELU_OP, ELU_OP = _register_ops()


def build_nc(debug=False):
    nc = bacc.Bacc("TRN2", target_bir_lowering=False)
    ht_d = nc.dram_tensor("ht", [BPC, F, N], f16, kind="ExternalInput")
    adjt_d = nc.dram_tensor("adjt", [BPC, N, N], i8, kind="ExternalInput")
    w_d = nc.dram_tensor("w", [BPC, F, F], f16, kind="ExternalInput")
    c_d = nc.dram_tensor("c12", [BPC, F, 2], f16, kind="ExternalInput")
    out_d = nc.dram_tensor("out", [BPC, F, N], f16, kind="ExternalOutput")

    with contextlib.ExitStack() as st:
        tc = st.enter_context(tile.TileContext(nc))
        const = st.enter_context(tc.tile_pool(name="const", bufs=1))
        arawp = st.enter_context(tc.tile_pool(name="araw", bufs=5))
        htp = st.enter_context(tc.tile_pool(name="ht", bufs=2))
        wa = st.enter_context(tc.tile_pool(name="wa", bufs=2))
        scp = st.enter_context(tc.tile_pool(name="sc", bufs=2))
        s1bp = st.enter_context(tc.tile_pool(name="s1b", bufs=2))
        uup = st.enter_context(tc.tile_pool(name="uu", bufs=4))
        ptp = st.enter_context(tc.tile_pool(name="pt", bufs=TRAIL0 + 3))
        vsp = st.enter_context(tc.tile_pool(name="vs", bufs=TRAIL0 + 3))
        zzp = st.enter_context(tc.tile_pool(name="zz", bufs=8))
        epp = st.enter_context(tc.tile_pool(name="ep", bufs=6))
        osp = st.enter_context(tc.tile_pool(name="os", bufs=3))
        whp = st.enter_context(tc.tile_pool(name="whs", bufs=NT + 1))
        psO = st.enter_context(tc.tile_pool(name="psO", bufs=8, space="PSUM"))

        negshift = const.tile([128, 1], f32)
        nc.vector.memset(negshift, -SHIFT)
        ones_f = const.tile([1, 128], f32)
        nc.vector.memset(ones_f, 1.0)
        ones_r = const.tile([1, 128], f32r)
        nc.vector.tensor_copy(ones_r, ones_f)

        state = {}

        # ---------- DMA emission (all input kicks on the sync engine)

        def kick_adjt(b, g):
            # one descriptor per 4 j-tiles: [128, 4, 2048] int8
            raw = arawp.tile([128, 4, N], i8, tag="araw", name=f"araw_{b}_{g}")
            nc.sync.dma_start(
                out=raw,
                in_=adjt_d[b, g * 512:(g + 1) * 512, :].rearrange(
                    "(q p) i -> p q i", p=128))
            state.setdefault((b, "raw"), {})[g] = raw

        def kick_h(b):
            hT = htp.tile([128, FT, N], f16, tag="ht", name=f"ht_{b}")
            nc.sync.dma_start(
                out=hT, in_=ht_d[b].rearrange("(ft p) n -> p ft n", p=128))
            state[b, "hT"] = hT

        def kick_c(b):
            csb = wa.tile([128, FT, 2], f16, tag="c", name=f"c_{b}")
            nc.sync.dma_start(
                out=csb, in_=c_d[b].rearrange("(ft p) k -> p ft k", p=128))
            state[b, "csb"] = csb

        def kick_w(b):
            w16 = wa.tile([128, FT, F], f16, tag="w", name=f"w_{b}")
            nc.sync.dma_start(
                out=w16, in_=w_d[b].rearrange("(ft p) o -> p ft o", p=128))
            state[b, "w16"] = w16

        # ---------- prep: sT (s1/s2 per j), s1b row broadcast, Wh
        # PSUM eviction copies ride the ACT engine (its head is idle
        # while DVE streams the fused units).

        def prep_sT(b, part):
            # sT[p, 2*it + k] = s_k[it*128 + p]  (contract f on partitions);
            # split so the first 4 units' s2 columns are ready early
            hT = state[b, "hT"]
            csb = state[b, "csb"]
            its = range(4) if part == 0 else range(4, NT)
            pst = psO.tile([128, 512], f32, tag="O", name=f"pst_{b}_{part}")
            for i, it in enumerate(its):
                for ft in range(FT):
                    nc.tensor.matmul(
                        pst[:, 2 * i:2 * i + 2],
                        hT[:, ft, it * 128:(it + 1) * 128],
                        csb[:, ft, :], start=(ft == 0), stop=(ft == FT - 1))
            sT = scp.tile([128, 2 * len(its)], f32, tag=f"st{part}",
                          name=f"st_{b}_{part}")
            nc.scalar.activation(out=sT, in_=pst[:, :2 * len(its)],
                                 func=AF.Copy, bias=0.0, scale=1.0)
            state[b, "sT", part] = sT

        def s2col(b, jt):
            if jt < 4:
                return state[b, "sT", 0][:, 2 * jt + 1:2 * jt + 2]
            return state[b, "sT", 1][:, 2 * (jt - 4) + 1:2 * (jt - 4) + 2]

        def prep_srow(b, ch):
            # s1 as rows: ps2[0, i] = s1[i-chunk]; broadcast via ones x s1row
            hT = state[b, "hT"]
            csb = state[b, "csb"]
            if ch == 0:
                state[b, "s1b"] = s1bp.tile([128, N], f32, tag="s1b",
                                            name=f"s1b_{b}")
            s1b = state[b, "s1b"]
            sl = slice(ch * 512, (ch + 1) * 512)
            ps2 = psO.tile([2, 512], f32, tag="O", name=f"ps2_{b}_{ch}")
            for ft in range(FT):
                nc.tensor.matmul(ps2, csb[:, ft, :], hT[:, ft, sl],
                                 start=(ft == 0), stop=(ft == FT - 1))
            s1row = scp.tile([1, 512], f32r, tag="s1r", bufs=2,
                             name=f"s1r_{b}_{ch}")
            nc.vector.tensor_copy(s1row, ps2[0:1, :])
            pb = psO.tile([128, 512], f32, tag="O", name=f"pb_{b}_{ch}")
            nc.tensor.matmul(pb, ones_r, s1row, start=True, stop=True)
            nc.scalar.activation(out=s1b[:, sl], in_=pb,
                                 func=AF.Copy, bias=0.0, scale=1.0)

        def prep_wh2(b, jp):
            # Wh for jt pair (2*jp, 2*jp+1) -> one [128, 512] copy
            hT = state[b, "hT"]
            w16 = state[b, "w16"]
            whs = state.setdefault((b, "wh"), {})
            pw = psO.tile([128, 512], f32, tag="O", name=f"pw_{b}_{jp}")
            for half in range(2):
                jt = 2 * jp + half
                for ft in range(FT):
                    nc.tensor.matmul(
                        pw[:, half * F:(half + 1) * F],
                        hT[:, ft, jt * 128:(jt + 1) * 128],
                        w16[:, ft, :], start=(ft == 0), stop=(ft == FT - 1))
            wh2 = whp.tile([128, 2 * F], f16, tag="wh", name=f"wh2_{b}_{jp}")
            nc.scalar.activation(out=wh2, in_=pw, func=AF.Copy,
                                 bias=0.0, scale=1.0)
            for half in range(2):
                whs[2 * jp + half] = wh2[:, half * F:(half + 1) * F]

        # ---------- stream unit: w2 = lrelu(s1 + s2 + adjm) -> exp -> V

        def unit(b, jt):
            raw = state[b, "raw"][jt // 4]
            adjm = raw[:, jt % 4, :]
            s1b = state[b, "s1b"]

            pts = state.setdefault((b, "pt"), {})
            pt = ptp.tile([128, N], f16, tag="pt", name=f"pt_{b}_{jt}")
            pts[jt] = pt
            # z accumulators pair up so one reciprocal serves two units
            jp, half = jt // 2, jt % 2
            if half == 0:
                state[b, "zp", jp] = zzp.tile([128, 2], f32, tag="z",
                                              bufs=4, name=f"z_{b}_{jp}")
            zp = state[b, "zp", jp]

            w2 = uup.tile([128, N], f16, tag="u", name=f"w2_{b}_{jt}")
            nc.vector._custom_dve(
                LRELU_OP, out=w2, in0=s1b, in1=adjm,
                s0=s2col(b, jt), s1=0.0, imm2=ALPHA)
            nc.scalar.activation(out=pt, in_=w2, func=AF.Exp,
                                 bias=negshift, scale=1.0,
                                 accum_out=zp[:, half:half + 1])

        def recip_v(b, jp):
            # emitted ~2 units after the pair's exps so the DVE never
            # head-of-line blocks on the ACT accumulator
            zr2 = zzp.tile([128, 2], f32, tag="zr", name=f"zr_{b}_{jp}")
            nc.vector.reciprocal(zr2, state[b, "zp", jp])
            for h in range(2):
                j2 = 2 * jp + h
                v = vsp.tile([128, F], f16, tag="v", name=f"v_{b}_{j2}")
                if j2 % V_ON_ACT_MOD == 0:
                    nc.scalar.activation(
                        out=v, in_=state[b, "wh"][j2], func=AF.Copy,
                        bias=0.0, scale=zr2[:, h:h + 1])
                else:
                    nc.vector.tensor_scalar_mul(
                        v, state[b, "wh"][j2], zr2[:, h:h + 1])
                state.setdefault((b, "v"), {})[j2] = v

        # ---------- PV: 8 psum tiles [2 ot x 4 ch], contract over jt

        def pv8(b, jt):
            pts = state[b, "pt"]
            vs = state[b, "v"]
            if jt == 0:
                pv = state.setdefault((b, "pvO"), {})
                for ot in range(FT):
                    for ch in range(4):
                        pv[ot * 4 + ch] = psO.tile(
                            [128, 512], f32, tag="O", name=f"O_{b}_{ot}_{ch}")
            Os = state[b, "pvO"]
            for ot in range(FT):
                for ch in range(4):
                    nc.tensor.matmul(
                        Os[ot * 4 + ch],
                        vs[jt][:, ot * 128:(ot + 1) * 128],
                        pts[jt][:, ch * 512:(ch + 1) * 512],
                        start=(jt == 0), stop=(jt == NT - 1))

        # ---------- elu epilogue: elu(x) = min(exp(x) - 1, relu(x))

        def ostage(b, ot):
            stg = state.setdefault((b, "ostg"), {})
            if ot not in stg:
                stg[ot] = osp.tile([128, N], f16, tag="os",
                                   name=f"os_{b}_{ot}")
            return stg[ot]

        def elu_tile(b, ot, ch):
            O = state[b, "pvO"][ot * 4 + ch]
            stg = ostage(b, ot)
            e1 = epp.tile([128, 512], f16, tag="e1", name=f"e1_{b}_{ot}_{ch}")
            nc.scalar.activation(out=e1, in_=O, func=AF.Exp,
                                 bias=0.0, scale=1.0 / VSCALE)
            nc.vector._custom_dve(
                ELU_OP, out=stg[:, ch * 512:(ch + 1) * 512],
                in0=e1, in1=O, s0=1.0 / VSCALE, s1=1.0)

        def flush_out(b, ot):
            nc.sync.dma_start(
                out=out_d[b, ot * 128:(ot + 1) * 128, :],
                in_=state[b, "ostg"][ot])

        # ---------- emission schedule (BPC == 2) ----------------------

        kick_c(0)
        kick_h(0)
        kick_w(0)
        kick_adjt(0, 0)
        kick_c(1)
        kick_h(1)
        kick_w(1)
        for g in range(1, 4):
            kick_adjt(0, g)

        # minimal batch-0 prep so the first fused unit launches asap:
        # the s1b broadcast and the first 4 s2 columns lead
        for ch in range(4):
            prep_srow(0, ch)
        prep_sT(0, 0)
        prep_wh2(0, 0)
        prep_sT(0, 1)
        for jp in range(1, 4):
            prep_wh2(0, jp)
        for g in range(4):
            kick_adjt(1, g)

        # stream batch 0; prep(1) (and wh2(0, 4..7)) drip into the first
        # units — all PSUM-allocating pieces land before pv8(0, 0), and
        # batch-1 pieces come late enough that their hT(1) dep is ready
        drip = {
            0: [lambda: prep_wh2(0, 4), lambda: prep_wh2(0, 5)],
            1: [lambda: prep_wh2(0, 6), lambda: prep_wh2(0, 7)],
            2: [lambda: prep_sT(1, 0)]
               + [lambda ch=ch: prep_srow(1, ch) for ch in range(2)],
            3: [lambda ch=ch: prep_srow(1, ch) for ch in range(2, 4)]
               + [lambda: prep_sT(1, 1)]
               + [lambda: prep_wh2(1, 0)],
            4: [lambda jp=jp: prep_wh2(1, jp) for jp in range(1, 8)],
        }
        for jt in range(NT):
            unit(0, jt)
            if jt >= 3 and jt % 2 == 1:
                recip_v(0, (jt - 3) // 2)
            for piece in drip.get(jt, ()):
                piece()
            if jt >= TRAIL0:
                pv8(0, jt - TRAIL0)
        recip_v(0, 7)
        for jt in range(NT - TRAIL0, NT):
            pv8(0, jt)

        # stream batch 1; elu(0) spread over the first 4 units so the
        # PSUM banks free before pv8(1, 0) allocates all 8
        for jt in range(NT):
            unit(1, jt)
            if jt >= 3 and jt % 2 == 1:
                recip_v(1, (jt - 3) // 2)
            if jt < 4:
                for ot in range(FT):
                    elu_tile(0, ot, jt)
                if jt == 3:
                    for ot in range(FT):
                        flush_out(0, ot)
            if jt >= TRAIL1:
                pv8(1, jt - TRAIL1)
        recip_v(1, 7)

        # tail: finish pv8(1) per (ot, ch) so elu(1) overlaps the PE
        pts1 = state[1, "pt"]
        vs1 = state[1, "v"]
        Os1 = state[1, "pvO"]
        for ot in range(FT):
            for ch in range(4):
                for jt in range(NT - TRAIL1, NT):
                    nc.tensor.matmul(
                        Os1[ot * 4 + ch],
                        vs1[jt][:, ot * 128:(ot + 1) * 128],
                        pts1[jt][:, ch * 512:(ch + 1) * 512],
                        start=False, stop=(jt == NT - 1))
                elu_tile(1, ot, ch)
            flush_out(1, ot)

    nc.compile()
    return nc


_NC_CACHE = {}


def _get_nc():
    if "nc" not in _NC_CACHE:
        _NC_CACHE["nc"] = build_nc()
    return _NC_CACHE["nc"]


def build_in_maps(h, adj, W, a):
    in_maps = []
    for c in range(NCORES):
        sl = slice(c * BPC, (c + 1) * BPC)
        adjm = ((adj[sl].transpose(0, 2, 1).astype(np.int16) - 1)
                * MASKC).astype(np.int8)
        ht = np.ascontiguousarray(
            h[sl].transpose(0, 2, 1)).astype(np.float16)
        w16 = (W[sl] * VSCALE).astype(np.float16)
        Fo = W.shape[-1]
        c12 = np.stack(
            [np.einsum('bfo,bo->bf', W[sl].astype(np.float64),
                       a[sl, :Fo, 0].astype(np.float64)),
             np.einsum('bfo,bo->bf', W[sl].astype(np.float64),
                       a[sl, Fo:, 0].astype(np.float64))],
            axis=-1).astype(np.float16)
        in_maps.append({
            "ht": ht,
            "adjt": np.ascontiguousarray(adjm),
            "w": np.ascontiguousarray(w16),
            "c12": np.ascontiguousarray(c12),
        })
    return in_maps


def kernel(h, adj, W, a):
    nc = _get_nc()
    res = run_bass_kernel_spmd(nc, build_in_maps(h, adj, W, a),
                               list(range(NCORES)))
    outs = [np.asarray(r["out"]) for r in res.results]   # each [BPC, F, N]
    full = np.concatenate(outs, axis=0)                  # [B, F, N]
    return np.ascontiguousarray(
        full.transpose(0, 2, 1)).astype(np.float32)


# revision 30
# speedup vs baseline: 1.0210x; 1.0210x over previous
"""GAT layer (nn_GAT_21930103013469) on 8 trn2 NeuronCores — v3.

Reference (per batch b):
    Wh  = h @ W                                   [N, F]
    s1  = Wh @ a1,  s2 = Wh @ a2                  [N]
    e   = leakyrelu(s1[:,None] + s2[None,:], 0.2) [N, N]
    att = softmax(where(adj>0, e, -9e15), axis=1)   (normalized over rows i)
    out = elu(att @ Wh)

Data parallel over B=16 (2 batches per core). Attention is computed
TRANSPOSED (PT[j, i], partition j, free i) so the softmax reduction
(over i) is the ACT Exp pass's accum_out and the output matmul
out^T[o, i] = sum_j V[j, o] * PT[j, i] contracts j on partitions.

v3 structure (vs the 218 us baseline):
  - adjT is HOST-prepped int8 in {-128, 0}; the mask rides the logit
    (u = s1[i] + s2[j] + adjm, exp suppresses masked entries by
    e^-25.6 after the leaky slope). 4x less DMA than int32.
  - per unit the whole logit assembly is ONE custom DVE op
        w2 = lrelu(s1b + adjm + s2col)
    followed by ONE ACT Exp (bias=-SHIFT, accum_out=z). The per-elem
    work is 1 DVE + 1 ACT pass (baseline: 1 DVE + 2 ACT equivalents).
  - h is HOST-transposed/cast to f16 (no PE transposes), W cast f16
    with VSCALE folded in, c = W @ a computed on host (F*F*2 MACs,
    0.1% of the flops).
  - elu epilogue: elu(x) = min(exp(x)-1, relu(x)): ACT exp + one
    fused DVE op; one output DMA per (batch, ot) row block.
  - Pool engine (ISA-limited to copies here) takes the PSUM->SBUF
    copies (wh, s1b, sT); sync engine takes all input DMA kicks.
"""
import sys

sys.path.insert(0, "/opt/trn_rl_repo")

import contextlib

import numpy as np

import concourse.bacc as bacc
import concourse.tile as tile
from concourse import mybir
from concourse.bass_utils import run_bass_kernel_spmd

B, N, F = 16, 2048, 256
NCORES = 8
BPC = B // NCORES          # batches per core
NT = N // 128              # 16 j tiles
FT = F // 128              # 2 fout tiles
ALPHA = 0.2
SHIFT = 10.0               # PT = exp(lrelu(u) - SHIFT)
VSCALE = 8.0               # folded into W on host; out = PV / VSCALE
MASKC = 128                # adjm = (adj - 1) * MASKC  in {-128, 0}
TRAIL0 = 7                 # pv trails pt production (batch 0): prep(1) is
                           # dripped into stream(0) and all its PSUM tiles
                           # must be emitted before pv8(0, 0) claims 8 banks
TRAIL1 = 4                 # batch 1 trails: elu(0) frees PSUM banks first;
                           # must stay >= 4 so v(1, jt-TRAIL1) is emitted
                           # (recip_v lags the exps by ~3 units)

f32, f32r, f16, i8 = (mybir.dt.float32, mybir.dt.float32r,
                      mybir.dt.float16, mybir.dt.int8)
AF = mybir.ActivationFunctionType
OP = mybir.AluOpType

# how many of the 32 V-scale passes go to ACT (balance DVE vs ACT)
V_ON_ACT_MOD = 8           # jt % MOD == 0 -> ACT copy-with-scale


# ---------------------------------------------------------------------------
# Custom fused DVE ops, registered at import into concourse.dve_ops'
# tables (same machinery as the production ops; the per-NEFF DVE table
# is generated from these specs by bass_utils.dve_table_for_ops).
# ---------------------------------------------------------------------------


def _register_ops():
    import concourse.dve_ops as dve_ops_mod
    from concourse.dve_ops import DveOp
    from concourse.dve_spec import C0, C1, C2, Spec, Src0, Src1
    from concourse.dve_spec import lower as dve_lower
    from concourse.dve_spec import maxx, minn, relu
    from concourse.dve_uop import DveOpSpec

    def mk(name, spec):
        for op in dve_ops_mod.OPS:
            if op.name == name:
                return op
        row = max(dve_ops_mod._SUB_OPCODE_FOR_NAME.values()) + 1
        shas = {
            ver: DveOpSpec(name=name, opcode=row,
                           uops=dve_lower(spec, ver=ver),
                           rd1_en=True).sha(ver)
            for ver in ("v3", "v4")
        }
        op = DveOp(name, spec, subdim=False, uops_sha=shas)
        dve_ops_mod.OPS.append(op)
        dve_ops_mod.CUSTOM_DVE_SPECS[name] = spec
        dve_ops_mod._SUB_OPCODE_FOR_NAME[name] = row
        return op

    # out = lrelu(in0 + in1 + s0), slope imm2
    y = (Src0 + Src1) + C0

    def lrelu_ref(in0, in1, s0, s1, imm2):
        yy = in0.astype(np.float32) + in1 + s0
        return np.maximum(yy, yy * imm2).astype(np.float32)

    lrelu_op = mk("LRELU_MADD_ANT",
                  Spec(body=maxx(y, y * C2), reference=lrelu_ref))

    # out = min(in0 - s1, relu(in1 * s0)):  elu(x) with in0 = exp(x),
    # in1 = VSCALE*x, s0 = 1/VSCALE, s1 = 1.
    def elu_ref(in0, in1, s0, s1, imm2):
        r = np.maximum(np.nan_to_num(in1.astype(np.float32) * s0,
                                     nan=0.0, posinf=np.inf,
                                     neginf=-np.inf), 0)
        return np.minimum(in0.astype(np.float32) - s1, r).astype(np.float32)

    elu_op = mk("ELU_TAIL_ANT",
                Spec(body=minn(Src0 - C1, relu(Src1 * C0)),
                     reference=elu_ref))
    return lrelu_op, elu_op


LRELU_OP, ELU_OP = _register_ops()


def build_nc(debug=False):
    nc = bacc.Bacc("TRN2", target_bir_lowering=False)
    ht_d = nc.dram_tensor("ht", [BPC, F, N], f16, kind="ExternalInput")
    adjt_d = nc.dram_tensor("adjt", [BPC, N, N], i8, kind="ExternalInput")
    w_d = nc.dram_tensor("w", [BPC, F, F], f16, kind="ExternalInput")
    c_d = nc.dram_tensor("c12", [BPC, F, 2], f16, kind="ExternalInput")
    out_d = nc.dram_tensor("out", [BPC, F, N], f16, kind="ExternalOutput")

    with contextlib.ExitStack() as st:
        tc = st.enter_context(tile.TileContext(nc))
        const = st.enter_context(tc.tile_pool(name="const", bufs=1))
        arawp = st.enter_context(tc.tile_pool(name="araw", bufs=4))
        htp = st.enter_context(tc.tile_pool(name="ht", bufs=2))
        wa = st.enter_context(tc.tile_pool(name="wa", bufs=2))
        scp = st.enter_context(tc.tile_pool(name="sc", bufs=2))
        s1bp = st.enter_context(tc.tile_pool(name="s1b", bufs=2))
        uup = st.enter_context(tc.tile_pool(name="uu", bufs=4))
        # deep enough that fused(1, 0) never waits on a pt slot while the
        # batch-0 PV tail drains on the PE
        ptp = st.enter_context(tc.tile_pool(name="pt", bufs=TRAIL0 + 5))
        vsp = st.enter_context(tc.tile_pool(name="vs", bufs=TRAIL0 + 3))
        zzp = st.enter_context(tc.tile_pool(name="zz", bufs=8))
        epp = st.enter_context(tc.tile_pool(name="ep", bufs=6))
        osp = st.enter_context(tc.tile_pool(name="os", bufs=3))
        whp = st.enter_context(tc.tile_pool(name="whs", bufs=NT + 1))
        psO = st.enter_context(tc.tile_pool(name="psO", bufs=8, space="PSUM"))

        negshift = const.tile([128, 1], f32)
        nc.vector.memset(negshift, -SHIFT)
        ones_f = const.tile([1, 128], f32)
        nc.vector.memset(ones_f, 1.0)
        ones_r = const.tile([1, 128], f32r)
        nc.vector.tensor_copy(ones_r, ones_f)

        state = {}

        # ---------- DMA emission (all input kicks on the sync engine)

        def kick_adjt(b, g):
            # one descriptor per 4 j-tiles: [128, 4, 2048] int8
            raw = arawp.tile([128, 4, N], i8, tag="araw", name=f"araw_{b}_{g}")
            nc.sync.dma_start(
                out=raw,
                in_=adjt_d[b, g * 512:(g + 1) * 512, :].rearrange(
                    "(q p) i -> p q i", p=128))
            state.setdefault((b, "raw"), {})[g] = raw

        def kick_h(b):
            hT = htp.tile([128, FT, N], f16, tag="ht", name=f"ht_{b}")
            nc.sync.dma_start(
                out=hT, in_=ht_d[b].rearrange("(ft p) n -> p ft n", p=128))
            state[b, "hT"] = hT

        def kick_c(b):
            csb = wa.tile([128, FT, 2], f16, tag="c", name=f"c_{b}")
            nc.sync.dma_start(
                out=csb, in_=c_d[b].rearrange("(ft p) k -> p ft k", p=128))
            state[b, "csb"] = csb

        def kick_w(b):
            w16 = wa.tile([128, FT, F], f16, tag="w", name=f"w_{b}")
            nc.sync.dma_start(
                out=w16, in_=w_d[b].rearrange("(ft p) o -> p ft o", p=128))
            state[b, "w16"] = w16

        # ---------- prep: sT (s1/s2 per j), s1b row broadcast, Wh
        # PSUM eviction copies ride the ACT engine (its head is idle
        # while DVE streams the fused units).

        def prep_sT(b, part):
            # sT[p, 2*it + k] = s_k[it*128 + p]  (contract f on partitions);
            # split so the first 4 units' s2 columns are ready early
            hT = state[b, "hT"]
            csb = state[b, "csb"]
            its = range(4) if part == 0 else range(4, NT)
            pst = psO.tile([128, 512], f32, tag="O", name=f"pst_{b}_{part}")
            for i, it in enumerate(its):
                for ft in range(FT):
                    nc.tensor.matmul(
                        pst[:, 2 * i:2 * i + 2],
                        hT[:, ft, it * 128:(it + 1) * 128],
                        csb[:, ft, :], start=(ft == 0), stop=(ft == FT - 1))
            sT = scp.tile([128, 2 * len(its)], f32, tag=f"st{part}",
                          name=f"st_{b}_{part}")
            nc.scalar.activation(out=sT, in_=pst[:, :2 * len(its)],
                                 func=AF.Copy, bias=0.0, scale=1.0)
            state[b, "sT", part] = sT

        def s2col(b, jt):
            if jt < 4:
                return state[b, "sT", 0][:, 2 * jt + 1:2 * jt + 2]
            return state[b, "sT", 1][:, 2 * (jt - 4) + 1:2 * (jt - 4) + 2]

        def prep_srow(b, ch):
            # s1 as rows: ps2[0, i] = s1[i-chunk]; broadcast via ones x s1row
            hT = state[b, "hT"]
            csb = state[b, "csb"]
            if ch == 0:
                state[b, "s1b"] = s1bp.tile([128, N], f32, tag="s1b",
                                            name=f"s1b_{b}")
            s1b = state[b, "s1b"]
            sl = slice(ch * 512, (ch + 1) * 512)
            ps2 = psO.tile([2, 512], f32, tag="O", name=f"ps2_{b}_{ch}")
            for ft in range(FT):
                nc.tensor.matmul(ps2, csb[:, ft, :], hT[:, ft, sl],
                                 start=(ft == 0), stop=(ft == FT - 1))
            s1row = scp.tile([1, 512], f32r, tag="s1r", bufs=2,
                             name=f"s1r_{b}_{ch}")
            nc.vector.tensor_copy(s1row, ps2[0:1, :])
            pb = psO.tile([128, 512], f32, tag="O", name=f"pb_{b}_{ch}")
            nc.tensor.matmul(pb, ones_r, s1row, start=True, stop=True)
            nc.scalar.activation(out=s1b[:, sl], in_=pb,
                                 func=AF.Copy, bias=0.0, scale=1.0)

        def prep_wh2(b, jp):
            # Wh for jt pair (2*jp, 2*jp+1) -> one [128, 512] copy
            hT = state[b, "hT"]
            w16 = state[b, "w16"]
            whs = state.setdefault((b, "wh"), {})
            pw = psO.tile([128, 512], f32, tag="O", name=f"pw_{b}_{jp}")
            for half in range(2):
                jt = 2 * jp + half
                for ft in range(FT):
                    nc.tensor.matmul(
                        pw[:, half * F:(half + 1) * F],
                        hT[:, ft, jt * 128:(jt + 1) * 128],
                        w16[:, ft, :], start=(ft == 0), stop=(ft == FT - 1))
            wh2 = whp.tile([128, 2 * F], f16, tag="wh", name=f"wh2_{b}_{jp}")
            nc.scalar.activation(out=wh2, in_=pw, func=AF.Copy,
                                 bias=0.0, scale=1.0)
            for half in range(2):
                whs[2 * jp + half] = wh2[:, half * F:(half + 1) * F]

        # ---------- stream unit: w2 = lrelu(s1 + s2 + adjm) -> exp -> V

        def unit(b, jt):
            raw = state[b, "raw"][jt // 4]
            adjm = raw[:, jt % 4, :]
            s1b = state[b, "s1b"]

            pts = state.setdefault((b, "pt"), {})
            pt = ptp.tile([128, N], f16, tag="pt", name=f"pt_{b}_{jt}")
            pts[jt] = pt
            # z accumulators pair up so one reciprocal serves two units
            jp, half = jt // 2, jt % 2
            if half == 0:
                state[b, "zp", jp] = zzp.tile([128, 2], f32, tag="z",
                                              bufs=4, name=f"z_{b}_{jp}")
            zp = state[b, "zp", jp]

            w2 = uup.tile([128, N], f16, tag="u", name=f"w2_{b}_{jt}")
            nc.vector._custom_dve(
                LRELU_OP, out=w2, in0=s1b, in1=adjm,
                s0=s2col(b, jt), s1=0.0, imm2=ALPHA)
            nc.scalar.activation(out=pt, in_=w2, func=AF.Exp,
                                 bias=negshift, scale=1.0,
                                 accum_out=zp[:, half:half + 1])

        def recip_v(b, jp):
            # emitted ~2 units after the pair's exps so the DVE never
            # head-of-line blocks on the ACT accumulator
            zr2 = zzp.tile([128, 2], f32, tag="zr", name=f"zr_{b}_{jp}")
            nc.vector.reciprocal(zr2, state[b, "zp", jp])
            for h in range(2):
                j2 = 2 * jp + h
                v = vsp.tile([128, F], f16, tag="v", name=f"v_{b}_{j2}")
                if j2 % V_ON_ACT_MOD == 0:
                    nc.scalar.activation(
                        out=v, in_=state[b, "wh"][j2], func=AF.Copy,
                        bias=0.0, scale=zr2[:, h:h + 1])
                else:
                    nc.vector.tensor_scalar_mul(
                        v, state[b, "wh"][j2], zr2[:, h:h + 1])
                state.setdefault((b, "v"), {})[j2] = v

        # ---------- PV: 8 psum tiles [2 ot x 4 ch], contract over jt

        def pv8(b, jt):
            pts = state[b, "pt"]
            vs = state[b, "v"]
            if jt == 0:
                pv = state.setdefault((b, "pvO"), {})
                for ot in range(FT):
                    for ch in range(4):
                        pv[ot * 4 + ch] = psO.tile(
                            [128, 512], f32, tag="O", name=f"O_{b}_{ot}_{ch}")
            Os = state[b, "pvO"]
            for ot in range(FT):
                for ch in range(4):
                    nc.tensor.matmul(
                        Os[ot * 4 + ch],
                        vs[jt][:, ot * 128:(ot + 1) * 128],
                        pts[jt][:, ch * 512:(ch + 1) * 512],
                        start=(jt == 0), stop=(jt == NT - 1))

        # ---------- elu epilogue: elu(x) = min(exp(x) - 1, relu(x))

        def ostage(b, ot):
            stg = state.setdefault((b, "ostg"), {})
            if ot not in stg:
                stg[ot] = osp.tile([128, N], f16, tag="os",
                                   name=f"os_{b}_{ot}")
            return stg[ot]

        def elu_tile(b, ot, ch):
            O = state[b, "pvO"][ot * 4 + ch]
            stg = ostage(b, ot)
            e1 = epp.tile([128, 512], f16, tag="e1", name=f"e1_{b}_{ot}_{ch}")
            nc.scalar.activation(out=e1, in_=O, func=AF.Exp,
                                 bias=0.0, scale=1.0 / VSCALE)
            nc.vector._custom_dve(
                ELU_OP, out=stg[:, ch * 512:(ch + 1) * 512],
                in0=e1, in1=O, s0=1.0 / VSCALE, s1=1.0)

        def flush_out(b, ot):
            nc.sync.dma_start(
                out=out_d[b, ot * 128:(ot + 1) * 128, :],
                in_=state[b, "ostg"][ot])

        # ---------- emission schedule (BPC == 2) ----------------------

        kick_c(0)
        kick_h(0)
        kick_w(0)
        kick_adjt(0, 0)
        kick_c(1)
        kick_h(1)
        kick_w(1)
        for g in range(1, 4):
            kick_adjt(0, g)

        # minimal batch-0 prep so the first fused unit launches asap:
        # the s1b broadcast and the first 4 s2 columns lead
        for ch in range(4):
            prep_srow(0, ch)
        prep_sT(0, 0)
        prep_wh2(0, 0)
        prep_sT(0, 1)
        for jp in range(1, 4):
            prep_wh2(0, jp)
        for g in range(4):
            kick_adjt(1, g)

        # stream batch 0; prep(1) (and wh2(0, 4..7)) drip into the first
        # units — all PSUM-allocating pieces land before pv8(0, 0), and
        # batch-1 pieces come late enough that their hT(1) dep is ready
        drip = {
            0: [lambda: prep_wh2(0, 4), lambda: prep_wh2(0, 5)],
            1: [lambda: prep_wh2(0, 6), lambda: prep_wh2(0, 7)],
            2: [lambda: prep_sT(1, 0)],
            3: [lambda ch=ch: prep_srow(1, ch) for ch in range(2)],
            4: [lambda ch=ch: prep_srow(1, ch) for ch in range(2, 4)]
               + [lambda: prep_sT(1, 1)],
            5: [lambda jp=jp: prep_wh2(1, jp) for jp in range(4)],
            6: [lambda jp=jp: prep_wh2(1, jp) for jp in range(4, 8)],
        }
        for jt in range(NT):
            unit(0, jt)
            if jt >= 2 and jt % 2 == 0:
                recip_v(0, (jt - 2) // 2)
            for piece in drip.get(jt, ()):
                piece()
            if jt >= TRAIL0:
                pv8(0, jt - TRAIL0)
        recip_v(0, 7)
        for jt in range(NT - TRAIL0, NT):
            pv8(0, jt)

        # stream batch 1; elu(0) spread over the first 4 units so the
        # PSUM banks free before pv8(1, 0) allocates all 8
        for jt in range(NT):
            unit(1, jt)
            if jt >= 2 and jt % 2 == 0:
                recip_v(1, (jt - 2) // 2)
            if jt < 4:
                for ot in range(FT):
                    elu_tile(0, ot, jt)
                if jt == 3:
                    for ot in range(FT):
                        flush_out(0, ot)
            if jt >= TRAIL1:
                pv8(1, jt - TRAIL1)
        recip_v(1, 7)

        # tail: finish pv8(1) per (ot, ch) so elu(1) overlaps the PE
        pts1 = state[1, "pt"]
        vs1 = state[1, "v"]
        Os1 = state[1, "pvO"]
        for ot in range(FT):
            for ch in range(4):
                for jt in range(NT - TRAIL1, NT):
                    nc.tensor.matmul(
                        Os1[ot * 4 + ch],
                        vs1[jt][:, ot * 128:(ot + 1) * 128],
                        pts1[jt][:, ch * 512:(ch + 1) * 512],
                        start=False, stop=(jt == NT - 1))
                elu_tile(1, ot, ch)
            flush_out(1, ot)

    nc.compile()
    return nc


_NC_CACHE = {}


def _get_nc():
    if "nc" not in _NC_CACHE:
        _NC_CACHE["nc"] = build_nc()
    return _NC_CACHE["nc"]


def build_in_maps(h, adj, W, a):
    in_maps = []
    for c in range(NCORES):
        sl = slice(c * BPC, (c + 1) * BPC)
        adjm = ((adj[sl].transpose(0, 2, 1).astype(np.int16) - 1)
                * MASKC).astype(np.int8)
        ht = np.ascontiguousarray(
            h[sl].transpose(0, 2, 1)).astype(np.float16)
        w16 = (W[sl] * VSCALE).astype(np.float16)
        Fo = W.shape[-1]
        c12 = np.stack(
            [np.einsum('bfo,bo->bf', W[sl].astype(np.float64),
                       a[sl, :Fo, 0].astype(np.float64)),
             np.einsum('bfo,bo->bf', W[sl].astype(np.float64),
                       a[sl, Fo:, 0].astype(np.float64))],
            axis=-1).astype(np.float16)
        in_maps.append({
            "ht": ht,
            "adjt": np.ascontiguousarray(adjm),
            "w": np.ascontiguousarray(w16),
            "c12": np.ascontiguousarray(c12),
        })
    return in_maps


def kernel(h, adj, W, a):
    nc = _get_nc()
    res = run_bass_kernel_spmd(nc, build_in_maps(h, adj, W, a),
                               list(range(NCORES)))
    outs = [np.asarray(r["out"]) for r in res.results]   # each [BPC, F, N]
    full = np.concatenate(outs, axis=0)                  # [B, F, N]
    return np.ascontiguousarray(
        full.transpose(0, 2, 1)).astype(np.float32)


# revision 35
# speedup vs baseline: 1.0367x; 1.0155x over previous
"""GAT layer (nn_GAT_21930103013469) on 8 trn2 NeuronCores — v3.

Reference (per batch b):
    Wh  = h @ W                                   [N, F]
    s1  = Wh @ a1,  s2 = Wh @ a2                  [N]
    e   = leakyrelu(s1[:,None] + s2[None,:], 0.2) [N, N]
    att = softmax(where(adj>0, e, -9e15), axis=1)   (normalized over rows i)
    out = elu(att @ Wh)

Data parallel over B=16 (2 batches per core). Attention is computed
TRANSPOSED (PT[j, i], partition j, free i) so the softmax reduction
(over i) is the ACT Exp pass's accum_out and the output matmul
out^T[o, i] = sum_j V[j, o] * PT[j, i] contracts j on partitions.

v3 structure (vs the 218 us baseline):
  - adjT is HOST-prepped int8 in {-128, 0}; the mask rides the logit
    (u = s1[i] + s2[j] + adjm, exp suppresses masked entries by
    e^-25.6 after the leaky slope). 4x less DMA than int32.
  - per unit the whole logit assembly is ONE custom DVE op
        w2 = lrelu(s1b + adjm + s2col)
    followed by ONE ACT Exp (bias=-SHIFT, accum_out=z). The per-elem
    work is 1 DVE + 1 ACT pass (baseline: 1 DVE + 2 ACT equivalents).
  - h is HOST-transposed/cast to f16 (no PE transposes), W cast f16
    with VSCALE folded in, c = W @ a computed on host (F*F*2 MACs,
    0.1% of the flops).
  - elu epilogue: elu(x) = min(exp(x)-1, relu(x)): ACT exp + one
    fused DVE op; one output DMA per (batch, ot) row block.
  - Pool engine (ISA-limited to copies here) takes the PSUM->SBUF
    copies (wh, s1b, sT); sync engine takes all input DMA kicks.
"""
import sys

sys.path.insert(0, "/opt/trn_rl_repo")

import contextlib

import numpy as np

import concourse.bacc as bacc
import concourse.tile as tile
from concourse import mybir
from concourse.bass_utils import run_bass_kernel_spmd

B, N, F = 16, 2048, 256
NCORES = 8
BPC = B // NCORES          # batches per core
NT = N // 128              # 16 j tiles
FT = F // 128              # 2 fout tiles
ALPHA = 0.2
SHIFT = 10.0               # PT = exp(lrelu(u) - SHIFT)
VSCALE = 8.0               # folded into W on host; out = PV / VSCALE
MASKC = 128                # adjm = (adj - 1) * MASKC  in {-128, 0}
TRAIL0 = 7                 # pv trails pt production (batch 0): prep(1) is
                           # dripped into stream(0) and all its PSUM tiles
                           # must be emitted before pv8(0, 0) claims 8 banks
TRAIL1 = 4                 # batch 1 trails: elu(0) frees PSUM banks first;
                           # must stay >= 4 so v(1, jt-TRAIL1) is emitted
                           # (recip_v lags the exps by ~3 units)

f32, f32r, f16, i8 = (mybir.dt.float32, mybir.dt.float32r,
                      mybir.dt.float16, mybir.dt.int8)
AF = mybir.ActivationFunctionType
OP = mybir.AluOpType

# how many of the 32 V-scale passes go to ACT (balance DVE vs ACT)
V_ON_ACT_MOD = 3           # jt % MOD == 0 -> ACT copy-with-scale


# ---------------------------------------------------------------------------
# Custom fused DVE ops, registered at import into concourse.dve_ops'
# tables (same machinery as the production ops; the per-NEFF DVE table
# is generated from these specs by bass_utils.dve_table_for_ops).
# ---------------------------------------------------------------------------


def _register_ops():
    import concourse.dve_ops as dve_ops_mod
    from concourse.dve_ops import DveOp
    from concourse.dve_spec import C0, C1, C2, Spec, Src0, Src1
    from concourse.dve_spec import lower as dve_lower
    from concourse.dve_spec import maxx, minn, relu
    from concourse.dve_uop import DveOpSpec

    def mk(name, spec):
        for op in dve_ops_mod.OPS:
            if op.name == name:
                return op
        row = max(dve_ops_mod._SUB_OPCODE_FOR_NAME.values()) + 1
        shas = {
            ver: DveOpSpec(name=name, opcode=row,
                           uops=dve_lower(spec, ver=ver),
                           rd1_en=True).sha(ver)
            for ver in ("v3", "v4")
        }
        op = DveOp(name, spec, subdim=False, uops_sha=shas)
        dve_ops_mod.OPS.append(op)
        dve_ops_mod.CUSTOM_DVE_SPECS[name] = spec
        dve_ops_mod._SUB_OPCODE_FOR_NAME[name] = row
        return op

    # out = lrelu(in0 + in1 + s0), slope imm2
    y = (Src0 + Src1) + C0

    def lrelu_ref(in0, in1, s0, s1, imm2):
        yy = in0.astype(np.float32) + in1 + s0
        return np.maximum(yy, yy * imm2).astype(np.float32)

    lrelu_op = mk("LRELU_MADD_ANT",
                  Spec(body=maxx(y, y * C2), reference=lrelu_ref))

    # out = min(in0 - s1, relu(in1 * s0)):  elu(x) with in0 = exp(x),
    # in1 = VSCALE*x, s0 = 1/VSCALE, s1 = 1.
    def elu_ref(in0, in1, s0, s1, imm2):
        r = np.maximum(np.nan_to_num(in1.astype(np.float32) * s0,
                                     nan=0.0, posinf=np.inf,
                                     neginf=-np.inf), 0)
        return np.minimum(in0.astype(np.float32) - s1, r).astype(np.float32)

    elu_op = mk("ELU_TAIL_ANT",
                Spec(body=minn(Src0 - C1, relu(Src1 * C0)),
                     reference=elu_ref))
    return lrelu_op, elu_op


LRELU_OP, ELU_OP = _register_ops()


def build_nc(debug=False):
    nc = bacc.Bacc("TRN2", target_bir_lowering=False)
    ht_d = nc.dram_tensor("ht", [BPC, F, N], f16, kind="ExternalInput")
    adjt_d = nc.dram_tensor("adjt", [BPC, N, N], i8, kind="ExternalInput")
    w_d = nc.dram_tensor("w", [BPC, F, F], f16, kind="ExternalInput")
    c_d = nc.dram_tensor("c12", [BPC, F, 2], f16, kind="ExternalInput")
    out_d = nc.dram_tensor("out", [BPC, F, N], f16, kind="ExternalOutput")

    with contextlib.ExitStack() as st:
        tc = st.enter_context(tile.TileContext(nc))
        const = st.enter_context(tc.tile_pool(name="const", bufs=1))
        arawp = st.enter_context(tc.tile_pool(name="araw", bufs=4))
        htp = st.enter_context(tc.tile_pool(name="ht", bufs=2))
        wa = st.enter_context(tc.tile_pool(name="wa", bufs=2))
        scp = st.enter_context(tc.tile_pool(name="sc", bufs=2))
        s1bp = st.enter_context(tc.tile_pool(name="s1b", bufs=2))
        uup = st.enter_context(tc.tile_pool(name="uu", bufs=4))
        # deep enough that fused(1, 0) never waits on a pt slot while the
        # batch-0 PV tail drains on the PE
        ptp = st.enter_context(tc.tile_pool(name="pt", bufs=TRAIL0 + 5))
        vsp = st.enter_context(tc.tile_pool(name="vs", bufs=TRAIL0 + 3))
        zzp = st.enter_context(tc.tile_pool(name="zz", bufs=8))
        epp = st.enter_context(tc.tile_pool(name="ep", bufs=6))
        osp = st.enter_context(tc.tile_pool(name="os", bufs=3))
        whp = st.enter_context(tc.tile_pool(name="whs", bufs=NT + 1))
        psO = st.enter_context(tc.tile_pool(name="psO", bufs=8, space="PSUM"))

        negshift = const.tile([128, 1], f32)
        nc.vector.memset(negshift, -SHIFT)

        state = {}

        # ---------- DMA emission (all input kicks on the sync engine)

        def kick_adjt(b, g):
            # one descriptor per 4 j-tiles: [128, 4, 2048] int8
            raw = arawp.tile([128, 4, N], i8, tag="araw", name=f"araw_{b}_{g}")
            nc.sync.dma_start(
                out=raw,
                in_=adjt_d[b, g * 512:(g + 1) * 512, :].rearrange(
                    "(q p) i -> p q i", p=128))
            state.setdefault((b, "raw"), {})[g] = raw

        def kick_h(b):
            hT = htp.tile([128, FT, N], f16, tag="ht", name=f"ht_{b}")
            nc.sync.dma_start(
                out=hT, in_=ht_d[b].rearrange("(ft p) n -> p ft n", p=128))
            state[b, "hT"] = hT

        def kick_c(b):
            csb = wa.tile([128, FT, 2], f16, tag="c", name=f"c_{b}")
            nc.sync.dma_start(
                out=csb, in_=c_d[b].rearrange("(ft p) k -> p ft k", p=128))
            state[b, "csb"] = csb

        def kick_w(b):
            w16 = wa.tile([128, FT, F], f16, tag="w", name=f"w_{b}")
            nc.sync.dma_start(
                out=w16, in_=w_d[b].rearrange("(ft p) o -> p ft o", p=128))
            state[b, "w16"] = w16

        # ---------- prep: sT (s1/s2 per j), s1b row broadcast, Wh
        # PSUM eviction copies ride the ACT engine (its head is idle
        # while DVE streams the fused units).

        def prep_sT(b, part):
            # sT[p, 2*it + k] = s_k[it*128 + p]  (contract f on partitions);
            # split so the first 4 units' s2 columns are ready early
            hT = state[b, "hT"]
            csb = state[b, "csb"]
            its = range(4) if part == 0 else range(4, NT)
            pst = psO.tile([128, 512], f32, tag="O", name=f"pst_{b}_{part}")
            for i, it in enumerate(its):
                for ft in range(FT):
                    nc.tensor.matmul(
                        pst[:, 2 * i:2 * i + 2],
                        hT[:, ft, it * 128:(it + 1) * 128],
                        csb[:, ft, :], start=(ft == 0), stop=(ft == FT - 1))
            sT = scp.tile([128, 2 * len(its)], f32, tag=f"st{part}",
                          name=f"st_{b}_{part}")
            nc.scalar.activation(out=sT, in_=pst[:, :2 * len(its)],
                                 func=AF.Copy, bias=0.0, scale=1.0)
            state[b, "sT", part] = sT

        def s2col(b, jt):
            if jt < 4:
                return state[b, "sT", 0][:, 2 * jt + 1:2 * jt + 2]
            return state[b, "sT", 1][:, 2 * (jt - 4) + 1:2 * (jt - 4) + 2]

        def prep_srow(b, ch):
            # s1 as rows: ps2[0, i] = s1[i-chunk]; broadcast via ones x s1row
            hT = state[b, "hT"]
            csb = state[b, "csb"]
            if ch == 0:
                state[b, "s1b"] = s1bp.tile([128, N], f32, tag="s1b",
                                            name=f"s1b_{b}")
            s1b = state[b, "s1b"]
            sl = slice(ch * 512, (ch + 1) * 512)
            ps2 = psO.tile([2, 512], f32, tag="O", name=f"ps2_{b}_{ch}")
            for ft in range(FT):
                nc.tensor.matmul(ps2, csb[:, ft, :], hT[:, ft, sl],
                                 start=(ft == 0), stop=(ft == FT - 1))
            s1row = scp.tile([1, 512], f32, tag="s1r", bufs=2,
                             name=f"s1r_{b}_{ch}")
            nc.vector.tensor_copy(s1row, ps2[0:1, :])
            # broadcast on the (otherwise idle) pool engine — saves the
            # rank-1 PE matmul and the ACT eviction copy
            nc.gpsimd.partition_broadcast(s1b[:, sl], s1row)

        def prep_wh2(b, jp):
            # Wh for jt pair (2*jp, 2*jp+1) -> one [128, 512] copy
            hT = state[b, "hT"]
            w16 = state[b, "w16"]
            whs = state.setdefault((b, "wh"), {})
            pw = psO.tile([128, 512], f32, tag="O", name=f"pw_{b}_{jp}")
            for half in range(2):
                jt = 2 * jp + half
                for ft in range(FT):
                    nc.tensor.matmul(
                        pw[:, half * F:(half + 1) * F],
                        hT[:, ft, jt * 128:(jt + 1) * 128],
                        w16[:, ft, :], start=(ft == 0), stop=(ft == FT - 1))
            wh2 = whp.tile([128, 2 * F], f16, tag="wh", name=f"wh2_{b}_{jp}")
            nc.scalar.activation(out=wh2, in_=pw, func=AF.Copy,
                                 bias=0.0, scale=1.0)
            for half in range(2):
                whs[2 * jp + half] = wh2[:, half * F:(half + 1) * F]

        # ---------- stream unit: w2 = lrelu(s1 + s2 + adjm) -> exp -> V

        def unit(b, jt):
            raw = state[b, "raw"][jt // 4]
            adjm = raw[:, jt % 4, :]
            s1b = state[b, "s1b"]

            pts = state.setdefault((b, "pt"), {})
            pt = ptp.tile([128, N], f16, tag="pt", name=f"pt_{b}_{jt}")
            pts[jt] = pt
            # z accumulators pair up so one reciprocal serves two units
            jp, half = jt // 2, jt % 2
            if half == 0:
                state[b, "zp", jp] = zzp.tile([128, 2], f32, tag="z",
                                              bufs=4, name=f"z_{b}_{jp}")
            zp = state[b, "zp", jp]

            w2 = uup.tile([128, N], f16, tag="u", name=f"w2_{b}_{jt}")
            nc.vector._custom_dve(
                LRELU_OP, out=w2, in0=s1b, in1=adjm,
                s0=s2col(b, jt), s1=0.0, imm2=ALPHA)
            nc.scalar.activation(out=pt, in_=w2, func=AF.Exp,
                                 bias=negshift, scale=1.0,
                                 accum_out=zp[:, half:half + 1])

        def recip_v(b, jp):
            # emitted ~2 units after the pair's exps so the DVE never
            # head-of-line blocks on the ACT accumulator
            zr2 = zzp.tile([128, 2], f32, tag="zr", name=f"zr_{b}_{jp}")
            nc.vector.reciprocal(zr2, state[b, "zp", jp])
            for h in range(2):
                j2 = 2 * jp + h
                v = vsp.tile([128, F], f16, tag="v", name=f"v_{b}_{j2}")
                if j2 % V_ON_ACT_MOD == 0:
                    nc.scalar.activation(
                        out=v, in_=state[b, "wh"][j2], func=AF.Copy,
                        bias=0.0, scale=zr2[:, h:h + 1])
                else:
                    nc.vector.tensor_scalar_mul(
                        v, state[b, "wh"][j2], zr2[:, h:h + 1])
                state.setdefault((b, "v"), {})[j2] = v

        # ---------- PV: 8 psum tiles [2 ot x 4 ch], contract over jt

        def pv8(b, jt):
            pts = state[b, "pt"]
            vs = state[b, "v"]
            if jt == 0:
                pv = state.setdefault((b, "pvO"), {})
                for ot in range(FT):
                    for ch in range(4):
                        pv[ot * 4 + ch] = psO.tile(
                            [128, 512], f32, tag="O", name=f"O_{b}_{ot}_{ch}")
            Os = state[b, "pvO"]
            for ot in range(FT):
                for ch in range(4):
                    nc.tensor.matmul(
                        Os[ot * 4 + ch],
                        vs[jt][:, ot * 128:(ot + 1) * 128],
                        pts[jt][:, ch * 512:(ch + 1) * 512],
                        start=(jt == 0), stop=(jt == NT - 1))

        # ---------- elu epilogue: elu(x) = min(exp(x) - 1, relu(x))

        def ostage(b, ot):
            stg = state.setdefault((b, "ostg"), {})
            if ot not in stg:
                stg[ot] = osp.tile([128, N], f16, tag="os",
                                   name=f"os_{b}_{ot}")
            return stg[ot]

        def elu_tile(b, ot, ch):
            O = state[b, "pvO"][ot * 4 + ch]
            stg = ostage(b, ot)
            e1 = epp.tile([128, 512], f16, tag="e1", name=f"e1_{b}_{ot}_{ch}")
            nc.scalar.activation(out=e1, in_=O, func=AF.Exp,
                                 bias=0.0, scale=1.0 / VSCALE)
            nc.vector._custom_dve(
                ELU_OP, out=stg[:, ch * 512:(ch + 1) * 512],
                in0=e1, in1=O, s0=1.0 / VSCALE, s1=1.0)

        def flush_out(b, ot):
            nc.sync.dma_start(
                out=out_d[b, ot * 128:(ot + 1) * 128, :],
                in_=state[b, "ostg"][ot])

        # ---------- emission schedule (BPC == 2) ----------------------

        kick_c(0)
        kick_h(0)
        kick_w(0)
        kick_adjt(0, 0)
        kick_c(1)
        kick_h(1)
        kick_w(1)
        for g in range(1, 4):
            kick_adjt(0, g)

        # minimal batch-0 prep so the first fused unit launches asap:
        # the s1b broadcast and the first 4 s2 columns lead
        for ch in range(4):
            prep_srow(0, ch)
        prep_sT(0, 0)
        prep_wh2(0, 0)
        prep_sT(0, 1)
        for jp in range(1, 4):
            prep_wh2(0, jp)
        for g in range(4):
            kick_adjt(1, g)

        # stream batch 0; prep(1) (and wh2(0, 4..7)) drip into the first
        # units — all PSUM-allocating pieces land before pv8(0, 0), and
        # batch-1 pieces come late enough that their hT(1) dep is ready
        drip = {
            0: [lambda: prep_wh2(0, 4), lambda: prep_wh2(0, 5)],
            1: [lambda: prep_wh2(0, 6), lambda: prep_wh2(0, 7)],
            2: [lambda: prep_sT(1, 0)],
            3: [lambda ch=ch: prep_srow(1, ch) for ch in range(2)],
            4: [lambda ch=ch: prep_srow(1, ch) for ch in range(2, 4)]
               + [lambda: prep_sT(1, 1)],
            5: [lambda jp=jp: prep_wh2(1, jp) for jp in range(4)],
            6: [lambda jp=jp: prep_wh2(1, jp) for jp in range(4, 8)],
        }
        for jt in range(NT):
            unit(0, jt)
            if jt >= 2 and jt % 2 == 0:
                recip_v(0, (jt - 2) // 2)
            for piece in drip.get(jt, ()):
                piece()
            if jt >= TRAIL0:
                pv8(0, jt - TRAIL0)
        # unit(1, 0) first: it is ready to run, so the DVE chews it while
        # recip_v(0, 7) waits for exp(0, 15)'s accumulator
        unit(1, 0)
        recip_v(0, 7)
        for jt in range(NT - TRAIL0, NT):
            pv8(0, jt)
        for ot in range(FT):
            elu_tile(0, ot, 0)

        # stream batch 1; elu(0) spread over the first 4 units so the
        # PSUM banks free before pv8(1, 0) allocates all 8
        for jt in range(1, NT):
            unit(1, jt)
            if jt >= 2 and jt % 2 == 0:
                recip_v(1, (jt - 2) // 2)
            if jt < 4:
                for ot in range(FT):
                    elu_tile(0, ot, jt)
                if jt == 3:
                    for ot in range(FT):
                        flush_out(0, ot)
            if jt >= TRAIL1:
                pv8(1, jt - TRAIL1)
        recip_v(1, 7)

        # tail: finish pv8(1) per (ot, ch) so elu(1) overlaps the PE
        pts1 = state[1, "pt"]
        vs1 = state[1, "v"]
        Os1 = state[1, "pvO"]
        for ot in range(FT):
            for ch in range(4):
                for jt in range(NT - TRAIL1, NT):
                    nc.tensor.matmul(
                        Os1[ot * 4 + ch],
                        vs1[jt][:, ot * 128:(ot + 1) * 128],
                        pts1[jt][:, ch * 512:(ch + 1) * 512],
                        start=False, stop=(jt == NT - 1))
                elu_tile(1, ot, ch)
            flush_out(1, ot)

    nc.compile()
    return nc


_NC_CACHE = {}


def _get_nc():
    if "nc" not in _NC_CACHE:
        _NC_CACHE["nc"] = build_nc()
    return _NC_CACHE["nc"]


def build_in_maps(h, adj, W, a):
    in_maps = []
    for c in range(NCORES):
        sl = slice(c * BPC, (c + 1) * BPC)
        adjm = ((adj[sl].transpose(0, 2, 1).astype(np.int16) - 1)
                * MASKC).astype(np.int8)
        ht = np.ascontiguousarray(
            h[sl].transpose(0, 2, 1)).astype(np.float16)
        w16 = (W[sl] * VSCALE).astype(np.float16)
        Fo = W.shape[-1]
        c12 = np.stack(
            [np.einsum('bfo,bo->bf', W[sl].astype(np.float64),
                       a[sl, :Fo, 0].astype(np.float64)),
             np.einsum('bfo,bo->bf', W[sl].astype(np.float64),
                       a[sl, Fo:, 0].astype(np.float64))],
            axis=-1).astype(np.float16)
        in_maps.append({
            "ht": ht,
            "adjt": np.ascontiguousarray(adjm),
            "w": np.ascontiguousarray(w16),
            "c12": np.ascontiguousarray(c12),
        })
    return in_maps


def kernel(h, adj, W, a):
    nc = _get_nc()
    res = run_bass_kernel_spmd(nc, build_in_maps(h, adj, W, a),
                               list(range(NCORES)))
    outs = [np.asarray(r["out"]) for r in res.results]   # each [BPC, F, N]
    full = np.concatenate(outs, axis=0)                  # [B, F, N]
    return np.ascontiguousarray(
        full.transpose(0, 2, 1)).astype(np.float32)


# revision 36
# speedup vs baseline: 1.0413x; 1.0044x over previous
"""GAT layer (nn_GAT_21930103013469) on 8 trn2 NeuronCores — v3.

Reference (per batch b):
    Wh  = h @ W                                   [N, F]
    s1  = Wh @ a1,  s2 = Wh @ a2                  [N]
    e   = leakyrelu(s1[:,None] + s2[None,:], 0.2) [N, N]
    att = softmax(where(adj>0, e, -9e15), axis=1)   (normalized over rows i)
    out = elu(att @ Wh)

Data parallel over B=16 (2 batches per core). Attention is computed
TRANSPOSED (PT[j, i], partition j, free i) so the softmax reduction
(over i) is the ACT Exp pass's accum_out and the output matmul
out^T[o, i] = sum_j V[j, o] * PT[j, i] contracts j on partitions.

v3 structure (vs the 218 us baseline):
  - adjT is HOST-prepped int8 in {-128, 0}; the mask rides the logit
    (u = s1[i] + s2[j] + adjm, exp suppresses masked entries by
    e^-25.6 after the leaky slope). 4x less DMA than int32.
  - per unit the whole logit assembly is ONE custom DVE op
        w2 = lrelu(s1b + adjm + s2col)
    followed by ONE ACT Exp (bias=-SHIFT, accum_out=z). The per-elem
    work is 1 DVE + 1 ACT pass (baseline: 1 DVE + 2 ACT equivalents).
  - h is HOST-transposed/cast to f16 (no PE transposes), W cast f16
    with VSCALE folded in, c = W @ a computed on host (F*F*2 MACs,
    0.1% of the flops).
  - elu epilogue: elu(x) = min(exp(x)-1, relu(x)): ACT exp + one
    fused DVE op; one output DMA per (batch, ot) row block.
  - Pool engine (ISA-limited to copies here) takes the PSUM->SBUF
    copies (wh, s1b, sT); sync engine takes all input DMA kicks.
"""
import sys

sys.path.insert(0, "/opt/trn_rl_repo")

import contextlib

import numpy as np

import concourse.bacc as bacc
import concourse.tile as tile
from concourse import mybir
from concourse.bass_utils import run_bass_kernel_spmd

B, N, F = 16, 2048, 256
NCORES = 8
BPC = B // NCORES          # batches per core
NT = N // 128              # 16 j tiles
FT = F // 128              # 2 fout tiles
ALPHA = 0.2
SHIFT = 10.0               # PT = exp(lrelu(u) - SHIFT)
VSCALE = 8.0               # folded into W on host; out = PV / VSCALE
MASKC = 128                # adjm = (adj - 1) * MASKC  in {-128, 0}
TRAIL0 = 7                 # pv trails pt production (batch 0): prep(1) is
                           # dripped into stream(0) and all its PSUM tiles
                           # must be emitted before pv8(0, 0) claims 8 banks
TRAIL1 = 2                 # batch 1 trails: elu(0) frees PSUM banks per-bank
                           # just ahead of pv8(1)'s writes; must stay >= 2 so
                           # v(1, jt-TRAIL1) is emitted (recip_v lags 2 units)

f32, f32r, f16, i8 = (mybir.dt.float32, mybir.dt.float32r,
                      mybir.dt.float16, mybir.dt.int8)
AF = mybir.ActivationFunctionType
OP = mybir.AluOpType

# how many of the 32 V-scale passes go to ACT (balance DVE vs ACT)
V_ON_ACT_MOD = 3           # jt % MOD == 0 -> ACT copy-with-scale


# ---------------------------------------------------------------------------
# Custom fused DVE ops, registered at import into concourse.dve_ops'
# tables (same machinery as the production ops; the per-NEFF DVE table
# is generated from these specs by bass_utils.dve_table_for_ops).
# ---------------------------------------------------------------------------


def _register_ops():
    import concourse.dve_ops as dve_ops_mod
    from concourse.dve_ops import DveOp
    from concourse.dve_spec import C0, C1, C2, Spec, Src0, Src1
    from concourse.dve_spec import lower as dve_lower
    from concourse.dve_spec import maxx, minn, relu
    from concourse.dve_uop import DveOpSpec

    def mk(name, spec):
        for op in dve_ops_mod.OPS:
            if op.name == name:
                return op
        row = max(dve_ops_mod._SUB_OPCODE_FOR_NAME.values()) + 1
        shas = {
            ver: DveOpSpec(name=name, opcode=row,
                           uops=dve_lower(spec, ver=ver),
                           rd1_en=True).sha(ver)
            for ver in ("v3", "v4")
        }
        op = DveOp(name, spec, subdim=False, uops_sha=shas)
        dve_ops_mod.OPS.append(op)
        dve_ops_mod.CUSTOM_DVE_SPECS[name] = spec
        dve_ops_mod._SUB_OPCODE_FOR_NAME[name] = row
        return op

    # out = lrelu(in0 + in1 + s0), slope imm2
    y = (Src0 + Src1) + C0

    def lrelu_ref(in0, in1, s0, s1, imm2):
        yy = in0.astype(np.float32) + in1 + s0
        return np.maximum(yy, yy * imm2).astype(np.float32)

    lrelu_op = mk("LRELU_MADD_ANT",
                  Spec(body=maxx(y, y * C2), reference=lrelu_ref))

    # out = min(in0 - s1, relu(in1 * s0)):  elu(x) with in0 = exp(x),
    # in1 = VSCALE*x, s0 = 1/VSCALE, s1 = 1.
    def elu_ref(in0, in1, s0, s1, imm2):
        r = np.maximum(np.nan_to_num(in1.astype(np.float32) * s0,
                                     nan=0.0, posinf=np.inf,
                                     neginf=-np.inf), 0)
        return np.minimum(in0.astype(np.float32) - s1, r).astype(np.float32)

    elu_op = mk("ELU_TAIL_ANT",
                Spec(body=minn(Src0 - C1, relu(Src1 * C0)),
                     reference=elu_ref))
    return lrelu_op, elu_op


LRELU_OP, ELU_OP = _register_ops()


def build_nc(debug=False):
    nc = bacc.Bacc("TRN2", target_bir_lowering=False)
    ht_d = nc.dram_tensor("ht", [BPC, F, N], f16, kind="ExternalInput")
    adjt_d = nc.dram_tensor("adjt", [BPC, N, N], i8, kind="ExternalInput")
    w_d = nc.dram_tensor("w", [BPC, F, F], f16, kind="ExternalInput")
    c_d = nc.dram_tensor("c12", [BPC, F, 2], f16, kind="ExternalInput")
    out_d = nc.dram_tensor("out", [BPC, F, N], f16, kind="ExternalOutput")

    with contextlib.ExitStack() as st:
        tc = st.enter_context(tile.TileContext(nc))
        const = st.enter_context(tc.tile_pool(name="const", bufs=1))
        arawp = st.enter_context(tc.tile_pool(name="araw", bufs=4))
        htp = st.enter_context(tc.tile_pool(name="ht", bufs=2))
        wa = st.enter_context(tc.tile_pool(name="wa", bufs=2))
        scp = st.enter_context(tc.tile_pool(name="sc", bufs=2))
        s1bp = st.enter_context(tc.tile_pool(name="s1b", bufs=2))
        uup = st.enter_context(tc.tile_pool(name="uu", bufs=4))
        # deep enough that fused(1, 0) never waits on a pt slot while the
        # batch-0 PV tail drains on the PE
        ptp = st.enter_context(tc.tile_pool(name="pt", bufs=TRAIL0 + 5))
        vsp = st.enter_context(tc.tile_pool(name="vs", bufs=TRAIL0 + 3))
        zzp = st.enter_context(tc.tile_pool(name="zz", bufs=8))
        epp = st.enter_context(tc.tile_pool(name="ep", bufs=6))
        osp = st.enter_context(tc.tile_pool(name="os", bufs=3))
        whp = st.enter_context(tc.tile_pool(name="whs", bufs=NT + 1))
        psO = st.enter_context(tc.tile_pool(name="psO", bufs=8, space="PSUM"))

        negshift = const.tile([128, 1], f32)
        nc.vector.memset(negshift, -SHIFT)

        state = {}

        # ---------- DMA emission (all input kicks on the sync engine)

        def kick_adjt(b, g):
            # one descriptor per 4 j-tiles: [128, 4, 2048] int8
            raw = arawp.tile([128, 4, N], i8, tag="araw", name=f"araw_{b}_{g}")
            nc.sync.dma_start(
                out=raw,
                in_=adjt_d[b, g * 512:(g + 1) * 512, :].rearrange(
                    "(q p) i -> p q i", p=128))
            state.setdefault((b, "raw"), {})[g] = raw

        def kick_h(b):
            hT = htp.tile([128, FT, N], f16, tag="ht", name=f"ht_{b}")
            nc.sync.dma_start(
                out=hT, in_=ht_d[b].rearrange("(ft p) n -> p ft n", p=128))
            state[b, "hT"] = hT

        def kick_c(b):
            csb = wa.tile([128, FT, 2], f16, tag="c", name=f"c_{b}")
            nc.sync.dma_start(
                out=csb, in_=c_d[b].rearrange("(ft p) k -> p ft k", p=128))
            state[b, "csb"] = csb

        def kick_w(b):
            w16 = wa.tile([128, FT, F], f16, tag="w", name=f"w_{b}")
            nc.sync.dma_start(
                out=w16, in_=w_d[b].rearrange("(ft p) o -> p ft o", p=128))
            state[b, "w16"] = w16

        # ---------- prep: sT (s1/s2 per j), s1b row broadcast, Wh
        # PSUM eviction copies ride the ACT engine (its head is idle
        # while DVE streams the fused units).

        def prep_sT(b, part):
            # sT[p, 2*it + k] = s_k[it*128 + p]  (contract f on partitions);
            # split so the first 4 units' s2 columns are ready early
            hT = state[b, "hT"]
            csb = state[b, "csb"]
            its = range(4) if part == 0 else range(4, NT)
            pst = psO.tile([128, 512], f32, tag="O", name=f"pst_{b}_{part}")
            for i, it in enumerate(its):
                for ft in range(FT):
                    nc.tensor.matmul(
                        pst[:, 2 * i:2 * i + 2],
                        hT[:, ft, it * 128:(it + 1) * 128],
                        csb[:, ft, :], start=(ft == 0), stop=(ft == FT - 1))
            sT = scp.tile([128, 2 * len(its)], f32, tag=f"st{part}",
                          name=f"st_{b}_{part}")
            nc.scalar.activation(out=sT, in_=pst[:, :2 * len(its)],
                                 func=AF.Copy, bias=0.0, scale=1.0)
            state[b, "sT", part] = sT

        def s2col(b, jt):
            if jt < 4:
                return state[b, "sT", 0][:, 2 * jt + 1:2 * jt + 2]
            return state[b, "sT", 1][:, 2 * (jt - 4) + 1:2 * (jt - 4) + 2]

        def prep_srow(b, ch):
            # s1 as rows: ps2[0, i] = s1[i-chunk]; broadcast via ones x s1row
            hT = state[b, "hT"]
            csb = state[b, "csb"]
            if ch == 0:
                state[b, "s1b"] = s1bp.tile([128, N], f32, tag="s1b",
                                            name=f"s1b_{b}")
            s1b = state[b, "s1b"]
            sl = slice(ch * 512, (ch + 1) * 512)
            ps2 = psO.tile([2, 512], f32, tag="O", name=f"ps2_{b}_{ch}")
            for ft in range(FT):
                nc.tensor.matmul(ps2, csb[:, ft, :], hT[:, ft, sl],
                                 start=(ft == 0), stop=(ft == FT - 1))
            s1row = scp.tile([1, 512], f32, tag="s1r", bufs=2,
                             name=f"s1r_{b}_{ch}")
            nc.vector.tensor_copy(s1row, ps2[0:1, :])
            # broadcast on the (otherwise idle) pool engine — saves the
            # rank-1 PE matmul and the ACT eviction copy
            nc.gpsimd.partition_broadcast(s1b[:, sl], s1row)

        def prep_wh2(b, jp):
            # Wh for jt pair (2*jp, 2*jp+1) -> one [128, 512] copy
            hT = state[b, "hT"]
            w16 = state[b, "w16"]
            whs = state.setdefault((b, "wh"), {})
            pw = psO.tile([128, 512], f32, tag="O", name=f"pw_{b}_{jp}")
            for half in range(2):
                jt = 2 * jp + half
                for ft in range(FT):
                    nc.tensor.matmul(
                        pw[:, half * F:(half + 1) * F],
                        hT[:, ft, jt * 128:(jt + 1) * 128],
                        w16[:, ft, :], start=(ft == 0), stop=(ft == FT - 1))
            wh2 = whp.tile([128, 2 * F], f16, tag="wh", name=f"wh2_{b}_{jp}")
            nc.scalar.activation(out=wh2, in_=pw, func=AF.Copy,
                                 bias=0.0, scale=1.0)
            for half in range(2):
                whs[2 * jp + half] = wh2[:, half * F:(half + 1) * F]

        # ---------- stream unit: w2 = lrelu(s1 + s2 + adjm) -> exp -> V

        def unit(b, jt):
            raw = state[b, "raw"][jt // 4]
            adjm = raw[:, jt % 4, :]
            s1b = state[b, "s1b"]

            pts = state.setdefault((b, "pt"), {})
            pt = ptp.tile([128, N], f16, tag="pt", name=f"pt_{b}_{jt}")
            pts[jt] = pt
            # z accumulators pair up so one reciprocal serves two units
            jp, half = jt // 2, jt % 2
            if half == 0:
                state[b, "zp", jp] = zzp.tile([128, 2], f32, tag="z",
                                              bufs=4, name=f"z_{b}_{jp}")
            zp = state[b, "zp", jp]

            w2 = uup.tile([128, N], f16, tag="u", name=f"w2_{b}_{jt}")
            nc.vector._custom_dve(
                LRELU_OP, out=w2, in0=s1b, in1=adjm,
                s0=s2col(b, jt), s1=0.0, imm2=ALPHA)
            nc.scalar.activation(out=pt, in_=w2, func=AF.Exp,
                                 bias=negshift, scale=1.0,
                                 accum_out=zp[:, half:half + 1])

        def recip_v(b, jp):
            # emitted ~2 units after the pair's exps so the DVE never
            # head-of-line blocks on the ACT accumulator
            zr2 = zzp.tile([128, 2], f32, tag="zr", name=f"zr_{b}_{jp}")
            nc.vector.reciprocal(zr2, state[b, "zp", jp])
            for h in range(2):
                j2 = 2 * jp + h
                v = vsp.tile([128, F], f16, tag="v", name=f"v_{b}_{j2}")
                if j2 % V_ON_ACT_MOD == 0:
                    nc.scalar.activation(
                        out=v, in_=state[b, "wh"][j2], func=AF.Copy,
                        bias=0.0, scale=zr2[:, h:h + 1])
                else:
                    nc.vector.tensor_scalar_mul(
                        v, state[b, "wh"][j2], zr2[:, h:h + 1])
                state.setdefault((b, "v"), {})[j2] = v

        # ---------- PV: 8 psum tiles [2 ot x 4 ch], contract over jt

        def pv8(b, jt):
            pts = state[b, "pt"]
            vs = state[b, "v"]
            if jt == 0:
                pv = state.setdefault((b, "pvO"), {})
                for ot in range(FT):
                    for ch in range(4):
                        pv[ot * 4 + ch] = psO.tile(
                            [128, 512], f32, tag="O", name=f"O_{b}_{ot}_{ch}")
            Os = state[b, "pvO"]
            for ot in range(FT):
                for ch in range(4):
                    nc.tensor.matmul(
                        Os[ot * 4 + ch],
                        vs[jt][:, ot * 128:(ot + 1) * 128],
                        pts[jt][:, ch * 512:(ch + 1) * 512],
                        start=(jt == 0), stop=(jt == NT - 1))

        # ---------- elu epilogue: elu(x) = min(exp(x) - 1, relu(x))

        def ostage(b, ot):
            stg = state.setdefault((b, "ostg"), {})
            if ot not in stg:
                stg[ot] = osp.tile([128, N], f16, tag="os",
                                   name=f"os_{b}_{ot}")
            return stg[ot]

        def elu_tile(b, ot, ch):
            O = state[b, "pvO"][ot * 4 + ch]
            stg = ostage(b, ot)
            e1 = epp.tile([128, 512], f16, tag="e1", name=f"e1_{b}_{ot}_{ch}")
            nc.scalar.activation(out=e1, in_=O, func=AF.Exp,
                                 bias=0.0, scale=1.0 / VSCALE)
            nc.vector._custom_dve(
                ELU_OP, out=stg[:, ch * 512:(ch + 1) * 512],
                in0=e1, in1=O, s0=1.0 / VSCALE, s1=1.0)

        def flush_out(b, ot):
            nc.sync.dma_start(
                out=out_d[b, ot * 128:(ot + 1) * 128, :],
                in_=state[b, "ostg"][ot])

        # ---------- emission schedule (BPC == 2) ----------------------

        kick_c(0)
        kick_h(0)
        kick_w(0)
        kick_adjt(0, 0)
        kick_c(1)
        kick_h(1)
        kick_w(1)
        for g in range(1, 4):
            kick_adjt(0, g)

        # minimal batch-0 prep so the first fused unit launches asap:
        # the s1b broadcast and the first 4 s2 columns lead
        for ch in range(4):
            prep_srow(0, ch)
        prep_sT(0, 0)
        prep_wh2(0, 0)
        prep_sT(0, 1)
        for jp in range(1, 4):
            prep_wh2(0, jp)
        for g in range(4):
            kick_adjt(1, g)

        # stream batch 0; prep(1) (and wh2(0, 4..7)) drip into the first
        # units — all PSUM-allocating pieces land before pv8(0, 0), and
        # batch-1 pieces come late enough that their hT(1) dep is ready
        drip = {
            0: [lambda: prep_wh2(0, 4), lambda: prep_wh2(0, 5)],
            1: [lambda: prep_wh2(0, 6), lambda: prep_wh2(0, 7)],
            2: [lambda: prep_sT(1, 0)],
            3: [lambda ch=ch: prep_srow(1, ch) for ch in range(2)],
            4: [lambda ch=ch: prep_srow(1, ch) for ch in range(2, 4)]
               + [lambda: prep_sT(1, 1)],
            5: [lambda jp=jp: prep_wh2(1, jp) for jp in range(4)],
            6: [lambda jp=jp: prep_wh2(1, jp) for jp in range(4, 8)],
        }
        for jt in range(NT):
            unit(0, jt)
            if jt >= 2 and jt % 2 == 0:
                recip_v(0, (jt - 2) // 2)
            for piece in drip.get(jt, ()):
                piece()
            if jt >= TRAIL0:
                pv8(0, jt - TRAIL0)
        # unit(1, 0) first: it is ready to run, so the DVE chews it while
        # recip_v(0, 7) waits for exp(0, 15)'s accumulator
        unit(1, 0)
        recip_v(0, 7)
        for jt in range(NT - TRAIL0, NT):
            pv8(0, jt)
        for ot in range(FT):
            elu_tile(0, ot, 0)

        # stream batch 1; elu(0) spread over the first 4 units so the
        # PSUM banks free before pv8(1, 0) allocates all 8
        for jt in range(1, NT):
            unit(1, jt)
            if jt >= 2 and jt % 2 == 0:
                recip_v(1, (jt - 2) // 2)
            if jt < 4:
                for ot in range(FT):
                    elu_tile(0, ot, jt)
                if jt == 3:
                    for ot in range(FT):
                        flush_out(0, ot)
            if jt >= TRAIL1:
                pv8(1, jt - TRAIL1)
        recip_v(1, 7)

        # tail: finish pv8(1) per (ot, ch) so elu(1) overlaps the PE
        pts1 = state[1, "pt"]
        vs1 = state[1, "v"]
        Os1 = state[1, "pvO"]
        for ot in range(FT):
            for ch in range(4):
                for jt in range(NT - TRAIL1, NT):
                    nc.tensor.matmul(
                        Os1[ot * 4 + ch],
                        vs1[jt][:, ot * 128:(ot + 1) * 128],
                        pts1[jt][:, ch * 512:(ch + 1) * 512],
                        start=False, stop=(jt == NT - 1))
                elu_tile(1, ot, ch)
            flush_out(1, ot)

    nc.compile()
    return nc


_NC_CACHE = {}


def _get_nc():
    if "nc" not in _NC_CACHE:
        _NC_CACHE["nc"] = build_nc()
    return _NC_CACHE["nc"]


def build_in_maps(h, adj, W, a):
    in_maps = []
    for c in range(NCORES):
        sl = slice(c * BPC, (c + 1) * BPC)
        adjm = ((adj[sl].transpose(0, 2, 1).astype(np.int16) - 1)
                * MASKC).astype(np.int8)
        ht = np.ascontiguousarray(
            h[sl].transpose(0, 2, 1)).astype(np.float16)
        w16 = (W[sl] * VSCALE).astype(np.float16)
        Fo = W.shape[-1]
        c12 = np.stack(
            [np.einsum('bfo,bo->bf', W[sl].astype(np.float64),
                       a[sl, :Fo, 0].astype(np.float64)),
             np.einsum('bfo,bo->bf', W[sl].astype(np.float64),
                       a[sl, Fo:, 0].astype(np.float64))],
            axis=-1).astype(np.float16)
        in_maps.append({
            "ht": ht,
            "adjt": np.ascontiguousarray(adjm),
            "w": np.ascontiguousarray(w16),
            "c12": np.ascontiguousarray(c12),
        })
    return in_maps


def kernel(h, adj, W, a):
    nc = _get_nc()
    res = run_bass_kernel_spmd(nc, build_in_maps(h, adj, W, a),
                               list(range(NCORES)))
    outs = [np.asarray(r["out"]) for r in res.results]   # each [BPC, F, N]
    full = np.concatenate(outs, axis=0)                  # [B, F, N]
    return np.ascontiguousarray(
        full.transpose(0, 2, 1)).astype(np.float32)


# revision 40
# speedup vs baseline: 1.0718x; 1.0293x over previous
"""GAT layer (nn_GAT_21930103013469) on 8 trn2 NeuronCores — v3.

Reference (per batch b):
    Wh  = h @ W                                   [N, F]
    s1  = Wh @ a1,  s2 = Wh @ a2                  [N]
    e   = leakyrelu(s1[:,None] + s2[None,:], 0.2) [N, N]
    att = softmax(where(adj>0, e, -9e15), axis=1)   (normalized over rows i)
    out = elu(att @ Wh)

Data parallel over B=16 (2 batches per core). Attention is computed
TRANSPOSED (PT[j, i], partition j, free i) so the softmax reduction
(over i) is the ACT Exp pass's accum_out and the output matmul
out^T[o, i] = sum_j V[j, o] * PT[j, i] contracts j on partitions.

v3 structure (vs the 218 us baseline):
  - adjT is HOST-prepped int8 in {-128, 0}; the mask rides the logit
    (u = s1[i] + s2[j] + adjm, exp suppresses masked entries by
    e^-25.6 after the leaky slope). 4x less DMA than int32.
  - per unit the whole logit assembly is ONE custom DVE op
        w2 = lrelu(s1b + adjm + s2col)
    followed by ONE ACT Exp (bias=-SHIFT, accum_out=z). The per-elem
    work is 1 DVE + 1 ACT pass (baseline: 1 DVE + 2 ACT equivalents).
  - h is HOST-transposed/cast to f16 (no PE transposes), W cast f16
    with VSCALE folded in, c = W @ a computed on host (F*F*2 MACs,
    0.1% of the flops).
  - elu epilogue: elu(x) = min(exp(x)-1, relu(x)): ACT exp + one
    fused DVE op; one output DMA per (batch, ot) row block.
  - Pool engine (ISA-limited to copies here) takes the PSUM->SBUF
    copies (wh, s1b, sT); sync engine takes all input DMA kicks.
"""
import sys

sys.path.insert(0, "/opt/trn_rl_repo")

import contextlib

import numpy as np

import concourse.bacc as bacc
import concourse.tile as tile
from concourse import mybir
from concourse.bass_utils import run_bass_kernel_spmd

B, N, F = 16, 2048, 256
NCORES = 8
BPC = B // NCORES          # batches per core
NT = N // 128              # 16 j tiles
FT = F // 128              # 2 fout tiles
ALPHA = 0.2
SHIFT = 10.0               # PT = exp(lrelu(u) - SHIFT)
VSCALE = 8.0               # folded into W on host; out = PV / VSCALE
MASKC = 128                # adjm = (adj - 1) * MASKC  in {-128, 0}
TRAIL0 = 7                 # pv trails pt production (batch 0): prep(1) is
                           # dripped into stream(0) and all its PSUM tiles
                           # must be emitted before pv8(0, 0) claims 8 banks
TRAIL1 = 2                 # batch 1 trails: elu(0) frees PSUM banks per-bank
                           # just ahead of pv8(1)'s writes; must stay >= 2 so
                           # v(1, jt-TRAIL1) is emitted (recip_v lags 2 units)

f32, f32r, f16, i8 = (mybir.dt.float32, mybir.dt.float32r,
                      mybir.dt.float16, mybir.dt.int8)
AF = mybir.ActivationFunctionType
OP = mybir.AluOpType

# how many of the 32 V-scale passes go to ACT (balance DVE vs ACT)
V_ON_ACT_MOD = 3           # jt % MOD == 0 -> ACT copy-with-scale


# ---------------------------------------------------------------------------
# Custom fused DVE ops, registered at import into concourse.dve_ops'
# tables (same machinery as the production ops; the per-NEFF DVE table
# is generated from these specs by bass_utils.dve_table_for_ops).
# ---------------------------------------------------------------------------


def _register_ops():
    import concourse.dve_ops as dve_ops_mod
    from concourse.dve_ops import DveOp
    from concourse.dve_spec import C0, C1, C2, Spec, Src0, Src1
    from concourse.dve_spec import lower as dve_lower
    from concourse.dve_spec import maxx, minn, relu
    from concourse.dve_uop import DveOpSpec

    def mk(name, spec):
        for op in dve_ops_mod.OPS:
            if op.name == name:
                return op
        row = max(dve_ops_mod._SUB_OPCODE_FOR_NAME.values()) + 1
        shas = {
            ver: DveOpSpec(name=name, opcode=row,
                           uops=dve_lower(spec, ver=ver),
                           rd1_en=True).sha(ver)
            for ver in ("v3", "v4")
        }
        op = DveOp(name, spec, subdim=False, uops_sha=shas)
        dve_ops_mod.OPS.append(op)
        dve_ops_mod.CUSTOM_DVE_SPECS[name] = spec
        dve_ops_mod._SUB_OPCODE_FOR_NAME[name] = row
        return op

    # out = lrelu(in0 + in1 + s0), slope imm2
    y = (Src0 + Src1) + C0

    def lrelu_ref(in0, in1, s0, s1, imm2):
        yy = in0.astype(np.float32) + in1 + s0
        return np.maximum(yy, yy * imm2).astype(np.float32)

    lrelu_op = mk("LRELU_MADD_ANT",
                  Spec(body=maxx(y, y * C2), reference=lrelu_ref))

    # out = min(in0 - s1, relu(in1 * s0)):  elu(x) with in0 = exp(x),
    # in1 = VSCALE*x, s0 = 1/VSCALE, s1 = 1.
    def elu_ref(in0, in1, s0, s1, imm2):
        r = np.maximum(np.nan_to_num(in1.astype(np.float32) * s0,
                                     nan=0.0, posinf=np.inf,
                                     neginf=-np.inf), 0)
        return np.minimum(in0.astype(np.float32) - s1, r).astype(np.float32)

    elu_op = mk("ELU_TAIL_ANT",
                Spec(body=minn(Src0 - C1, relu(Src1 * C0)),
                     reference=elu_ref))
    return lrelu_op, elu_op


LRELU_OP, ELU_OP = _register_ops()


def build_nc(debug=False):
    nc = bacc.Bacc("TRN2", target_bir_lowering=False)
    ht_d = nc.dram_tensor("ht", [BPC, F, N], f16, kind="ExternalInput")
    adjt_d = nc.dram_tensor("adjt", [BPC, N, N], i8, kind="ExternalInput")
    w_d = nc.dram_tensor("w", [BPC, F, F], f16, kind="ExternalInput")
    c_d = nc.dram_tensor("c12", [BPC, F, 2], f16, kind="ExternalInput")
    out_d = nc.dram_tensor("out", [BPC, F, N], f16, kind="ExternalOutput")

    with contextlib.ExitStack() as st:
        tc = st.enter_context(tile.TileContext(nc))
        const = st.enter_context(tc.tile_pool(name="const", bufs=1))
        arawp = st.enter_context(tc.tile_pool(name="araw", bufs=4))
        htp = st.enter_context(tc.tile_pool(name="ht", bufs=2))
        wa = st.enter_context(tc.tile_pool(name="wa", bufs=2))
        scp = st.enter_context(tc.tile_pool(name="sc", bufs=2))
        s1bp = st.enter_context(tc.tile_pool(name="s1b", bufs=2))
        uup = st.enter_context(tc.tile_pool(name="uu", bufs=4))
        # deep enough that fused(1, 0) never waits on a pt slot while the
        # batch-0 PV tail drains on the PE
        ptp = st.enter_context(tc.tile_pool(name="pt", bufs=TRAIL0 + 5))
        vsp = st.enter_context(tc.tile_pool(name="vs", bufs=TRAIL0 + 3))
        zzp = st.enter_context(tc.tile_pool(name="zz", bufs=8))
        epp = st.enter_context(tc.tile_pool(name="ep", bufs=6))
        osp = st.enter_context(tc.tile_pool(name="os", bufs=3))
        whp = st.enter_context(tc.tile_pool(name="whs", bufs=NT + 1))
        psO = st.enter_context(tc.tile_pool(name="psO", bufs=8, space="PSUM"))

        negshift = const.tile([128, 1], f32)
        nc.vector.memset(negshift, -SHIFT)

        state = {}

        # ---------- DMA emission (all input kicks on the sync engine)

        def kick_adjt(b, g):
            # one descriptor per 4 j-tiles: [128, 4, 2048] int8
            raw = arawp.tile([128, 4, N], i8, tag="araw", name=f"araw_{b}_{g}")
            nc.sync.dma_start(
                out=raw,
                in_=adjt_d[b, g * 512:(g + 1) * 512, :].rearrange(
                    "(q p) i -> p q i", p=128))
            state.setdefault((b, "raw"), {})[g] = raw

        def kick_h(b):
            # 4 column-chunk descriptors so the s1/s2 prep chain can start
            # on chunk 0 while the rest of hT is still in flight
            chunks = []
            for c in range(4):
                hc = htp.tile([128, FT, 512], f16, tag=f"ht{c}",
                              name=f"ht_{b}_{c}")
                nc.sync.dma_start(
                    out=hc,
                    in_=ht_d[b, :, c * 512:(c + 1) * 512].rearrange(
                        "(ft p) n -> p ft n", p=128))
                chunks.append(hc)
            state[b, "hT"] = chunks

        def ht_cols(b, ft, lo, width):
            c = lo // 512
            assert lo + width <= (c + 1) * 512
            return state[b, "hT"][c][:, ft, lo - c * 512:lo - c * 512 + width]

        def kick_c(b):
            csb = wa.tile([128, FT, 2], f16, tag="c", name=f"c_{b}")
            nc.sync.dma_start(
                out=csb, in_=c_d[b].rearrange("(ft p) k -> p ft k", p=128))
            state[b, "csb"] = csb

        def kick_w(b):
            w16 = wa.tile([128, FT, F], f16, tag="w", name=f"w_{b}")
            nc.sync.dma_start(
                out=w16, in_=w_d[b].rearrange("(ft p) o -> p ft o", p=128))
            state[b, "w16"] = w16

        # ---------- prep: sT (s1/s2 per j), s1b row broadcast, Wh
        # PSUM eviction copies ride the ACT engine (its head is idle
        # while DVE streams the fused units).

        def prep_sT(b, part):
            # sT[p, 2*it + k] = s_k[it*128 + p]  (contract f on partitions);
            # split so the first 4 units' s2 columns are ready early
            csb = state[b, "csb"]
            its = range(4) if part == 0 else range(4, NT)
            pst = psO.tile([128, 512], f32, tag="O", name=f"pst_{b}_{part}")
            for i, it in enumerate(its):
                for ft in range(FT):
                    nc.tensor.matmul(
                        pst[:, 2 * i:2 * i + 2],
                        ht_cols(b, ft, it * 128, 128),
                        csb[:, ft, :], start=(ft == 0), stop=(ft == FT - 1))
            sT = scp.tile([128, 2 * len(its)], f32, tag=f"st{part}",
                          name=f"st_{b}_{part}")
            nc.scalar.activation(out=sT, in_=pst[:, :2 * len(its)],
                                 func=AF.Copy, bias=0.0, scale=1.0)
            state[b, "sT", part] = sT

        def s2col(b, jt):
            if jt < 4:
                return state[b, "sT", 0][:, 2 * jt + 1:2 * jt + 2]
            return state[b, "sT", 1][:, 2 * (jt - 4) + 1:2 * (jt - 4) + 2]

        def prep_srow(b, ch):
            # s1 as rows: ps2[0, i] = s1[i-chunk]; broadcast to s1b
            csb = state[b, "csb"]
            if ch == 0:
                state[b, "s1b"] = s1bp.tile([128, N], f32, tag="s1b",
                                            name=f"s1b_{b}")
            s1b = state[b, "s1b"]
            sl = slice(ch * 512, (ch + 1) * 512)
            ps2 = psO.tile([2, 512], f32, tag="O", name=f"ps2_{b}_{ch}")
            for ft in range(FT):
                nc.tensor.matmul(ps2, csb[:, ft, :],
                                 ht_cols(b, ft, ch * 512, 512),
                                 start=(ft == 0), stop=(ft == FT - 1))
            s1row = scp.tile([1, 512], f32, tag="s1r", bufs=2,
                             name=f"s1r_{b}_{ch}")
            nc.vector.tensor_copy(s1row, ps2[0:1, :])
            # broadcast on the (otherwise idle) pool engine — saves the
            # rank-1 PE matmul and the ACT eviction copy
            nc.gpsimd.partition_broadcast(s1b[:, sl], s1row)

        def prep_wh2(b, jp):
            # Wh for jt pair (2*jp, 2*jp+1) -> one [128, 512] copy
            w16 = state[b, "w16"]
            whs = state.setdefault((b, "wh"), {})
            pw = psO.tile([128, 512], f32, tag="O", name=f"pw_{b}_{jp}")
            for half in range(2):
                jt = 2 * jp + half
                for ft in range(FT):
                    nc.tensor.matmul(
                        pw[:, half * F:(half + 1) * F],
                        ht_cols(b, ft, jt * 128, 128),
                        w16[:, ft, :], start=(ft == 0), stop=(ft == FT - 1))
            wh2 = whp.tile([128, 2 * F], f16, tag="wh", name=f"wh2_{b}_{jp}")
            nc.scalar.activation(out=wh2, in_=pw, func=AF.Copy,
                                 bias=0.0, scale=1.0)
            for half in range(2):
                whs[2 * jp + half] = wh2[:, half * F:(half + 1) * F]

        # ---------- stream unit: w2 = lrelu(s1 + s2 + adjm) -> exp -> V

        def unit(b, jt):
            raw = state[b, "raw"][jt // 4]
            adjm = raw[:, jt % 4, :]
            s1b = state[b, "s1b"]

            pts = state.setdefault((b, "pt"), {})
            pt = ptp.tile([128, N], f16, tag="pt", name=f"pt_{b}_{jt}")
            pts[jt] = pt
            # z accumulators pair up so one reciprocal serves two units
            jp, half = jt // 2, jt % 2
            if half == 0:
                state[b, "zp", jp] = zzp.tile([128, 2], f32, tag="z",
                                              bufs=4, name=f"z_{b}_{jp}")
            zp = state[b, "zp", jp]

            w2 = uup.tile([128, N], f16, tag="u", name=f"w2_{b}_{jt}")
            nc.vector._custom_dve(
                LRELU_OP, out=w2, in0=s1b, in1=adjm,
                s0=s2col(b, jt), s1=0.0, imm2=ALPHA)
            nc.scalar.activation(out=pt, in_=w2, func=AF.Exp,
                                 bias=negshift, scale=1.0,
                                 accum_out=zp[:, half:half + 1])

        def recip_v(b, jp):
            # emitted ~2 units after the pair's exps so the DVE never
            # head-of-line blocks on the ACT accumulator
            zr2 = zzp.tile([128, 2], f32, tag="zr", name=f"zr_{b}_{jp}")
            nc.vector.reciprocal(zr2, state[b, "zp", jp])
            for h in range(2):
                j2 = 2 * jp + h
                v = vsp.tile([128, F], f16, tag="v", name=f"v_{b}_{j2}")
                if j2 % V_ON_ACT_MOD == 0:
                    nc.scalar.activation(
                        out=v, in_=state[b, "wh"][j2], func=AF.Copy,
                        bias=0.0, scale=zr2[:, h:h + 1])
                else:
                    nc.vector.tensor_scalar_mul(
                        v, state[b, "wh"][j2], zr2[:, h:h + 1])
                state.setdefault((b, "v"), {})[j2] = v

        # ---------- PV: 8 psum tiles [2 ot x 4 ch], contract over jt

        def pv8(b, jt):
            pts = state[b, "pt"]
            vs = state[b, "v"]
            if jt == 0:
                pv = state.setdefault((b, "pvO"), {})
                for ot in range(FT):
                    for ch in range(4):
                        pv[ot * 4 + ch] = psO.tile(
                            [128, 512], f32, tag="O", name=f"O_{b}_{ot}_{ch}")
            Os = state[b, "pvO"]
            for ot in range(FT):
                for ch in range(4):
                    nc.tensor.matmul(
                        Os[ot * 4 + ch],
                        vs[jt][:, ot * 128:(ot + 1) * 128],
                        pts[jt][:, ch * 512:(ch + 1) * 512],
                        start=(jt == 0), stop=(jt == NT - 1))

        # ---------- elu epilogue: elu(x) = min(exp(x) - 1, relu(x))

        def ostage(b, ot):
            stg = state.setdefault((b, "ostg"), {})
            if ot not in stg:
                stg[ot] = osp.tile([128, N], f16, tag="os",
                                   name=f"os_{b}_{ot}")
            return stg[ot]

        def elu_tile(b, ot, ch):
            O = state[b, "pvO"][ot * 4 + ch]
            stg = ostage(b, ot)
            e1 = epp.tile([128, 512], f16, tag="e1", name=f"e1_{b}_{ot}_{ch}")
            nc.scalar.activation(out=e1, in_=O, func=AF.Exp,
                                 bias=0.0, scale=1.0 / VSCALE)
            nc.vector._custom_dve(
                ELU_OP, out=stg[:, ch * 512:(ch + 1) * 512],
                in0=e1, in1=O, s0=1.0 / VSCALE, s1=1.0)

        def flush_out(b, ot):
            nc.sync.dma_start(
                out=out_d[b, ot * 128:(ot + 1) * 128, :],
                in_=state[b, "ostg"][ot])

        # ---------- emission schedule (BPC == 2) ----------------------

        kick_c(0)
        kick_h(0)
        kick_w(0)
        kick_adjt(0, 0)
        kick_c(1)
        kick_h(1)
        kick_w(1)
        for g in range(1, 4):
            kick_adjt(0, g)

        # minimal batch-0 prep so the first fused unit launches asap:
        # the s1b broadcast and the first 4 s2 columns lead
        for ch in range(4):
            prep_srow(0, ch)
        prep_sT(0, 0)
        prep_wh2(0, 0)
        prep_sT(0, 1)
        for jp in range(1, 4):
            prep_wh2(0, jp)
        for g in range(4):
            kick_adjt(1, g)

        # stream batch 0; prep(1) (and wh2(0, 4..7)) drip into the first
        # units — all PSUM-allocating pieces land before pv8(0, 0), and
        # batch-1 pieces come late enough that their hT(1) dep is ready
        drip = {
            0: [lambda: prep_wh2(0, 4), lambda: prep_wh2(0, 5)],
            1: [lambda: prep_wh2(0, 6), lambda: prep_wh2(0, 7)],
            2: [lambda: prep_sT(1, 0)],
            3: [lambda ch=ch: prep_srow(1, ch) for ch in range(2)],
            4: [lambda ch=ch: prep_srow(1, ch) for ch in range(2, 4)]
               + [lambda: prep_sT(1, 1)],
            5: [lambda jp=jp: prep_wh2(1, jp) for jp in range(4)],
            6: [lambda jp=jp: prep_wh2(1, jp) for jp in range(4, 8)],
        }
        for jt in range(NT):
            unit(0, jt)
            if jt >= 2 and jt % 2 == 0:
                recip_v(0, (jt - 2) // 2)
            for piece in drip.get(jt, ()):
                piece()
            if jt >= TRAIL0:
                pv8(0, jt - TRAIL0)
        # unit(1, 0) first: it is ready to run, so the DVE chews it while
        # recip_v(0, 7) waits for exp(0, 15)'s accumulator
        unit(1, 0)
        recip_v(0, 7)
        for jt in range(NT - TRAIL0, NT):
            pv8(0, jt)
        for ot in range(FT):
            elu_tile(0, ot, 0)

        # stream batch 1; elu(0) spread over the first 4 units so the
        # PSUM banks free before pv8(1, 0) allocates all 8
        for jt in range(1, NT):
            unit(1, jt)
            if jt >= 2 and jt % 2 == 0:
                recip_v(1, (jt - 2) // 2)
            if jt < 4:
                for ot in range(FT):
                    elu_tile(0, ot, jt)
                if jt == 3:
                    for ot in range(FT):
                        flush_out(0, ot)
            if jt >= TRAIL1:
                pv8(1, jt - TRAIL1)
        recip_v(1, 7)

        # tail: finish pv8(1) per (ot, ch) so elu(1) overlaps the PE
        pts1 = state[1, "pt"]
        vs1 = state[1, "v"]
        Os1 = state[1, "pvO"]
        for ot in range(FT):
            for ch in range(4):
                for jt in range(NT - TRAIL1, NT):
                    nc.tensor.matmul(
                        Os1[ot * 4 + ch],
                        vs1[jt][:, ot * 128:(ot + 1) * 128],
                        pts1[jt][:, ch * 512:(ch + 1) * 512],
                        start=False, stop=(jt == NT - 1))
                elu_tile(1, ot, ch)
            flush_out(1, ot)

    nc.compile()
    return nc


_NC_CACHE = {}


def _get_nc():
    if "nc" not in _NC_CACHE:
        _NC_CACHE["nc"] = build_nc()
    return _NC_CACHE["nc"]


def build_in_maps(h, adj, W, a):
    in_maps = []
    for c in range(NCORES):
        sl = slice(c * BPC, (c + 1) * BPC)
        adjm = ((adj[sl].transpose(0, 2, 1).astype(np.int16) - 1)
                * MASKC).astype(np.int8)
        ht = np.ascontiguousarray(
            h[sl].transpose(0, 2, 1)).astype(np.float16)
        w16 = (W[sl] * VSCALE).astype(np.float16)
        Fo = W.shape[-1]
        c12 = np.stack(
            [np.einsum('bfo,bo->bf', W[sl].astype(np.float64),
                       a[sl, :Fo, 0].astype(np.float64)),
             np.einsum('bfo,bo->bf', W[sl].astype(np.float64),
                       a[sl, Fo:, 0].astype(np.float64))],
            axis=-1).astype(np.float16)
        in_maps.append({
            "ht": ht,
            "adjt": np.ascontiguousarray(adjm),
            "w": np.ascontiguousarray(w16),
            "c12": np.ascontiguousarray(c12),
        })
    return in_maps


def kernel(h, adj, W, a):
    nc = _get_nc()
    res = run_bass_kernel_spmd(nc, build_in_maps(h, adj, W, a),
                               list(range(NCORES)))
    outs = [np.asarray(r["out"]) for r in res.results]   # each [BPC, F, N]
    full = np.concatenate(outs, axis=0)                  # [B, F, N]
    return np.ascontiguousarray(
        full.transpose(0, 2, 1)).astype(np.float32)
